# revision 30
# baseline (speedup 1.0000x reference)
"""Trainium2 Bass kernel for CumsumAttention (v3).

Full-input contract: kernel(**inputs) takes the complete (unsharded) inputs
and returns the full [B, T, C] float32 output. Internally the work is
data-parallel over the batch dimension across 8 NeuronCores (2 batches per
core); each core runs the same Bass/Tile program on its own batch shard.

Math (per batch, reference semantics):
  k = x @ Wk.T ; q = x @ Wq.T ; v = x @ Wv.T   (biases all zero here)
  angle[t] = sum_{s>=t} ang_emb[idx[s]]        (reverse cumsum over T)
  rot/inv_rot: per channel-pair rotation by angle
  wei  = softplus((rot(k) @ rot(q).T) / sqrt(C)) masked causally (s <= t)
  out  = inv_rot(wei @ rot(v)) @ Wp.T

Engine plan (vs the 337.5us fp16 baseline):
  - Logits matmul in fp8e4 DoubleRow (2 K-tiles per instruction, 2x MAC
    rate); rot(k)/rot(q) written to fp8 by the DVE rotation.  Measured
    rel err ~1.3e-2 < 2e-2.
  - v-projection accumulates ci-outer across PSUM-resident groups so
    compute starts on the first x/Wv chunk; the angle-cumsum PE chain is
    interleaved one tile per chunk so its carry round-trips hide.
  - PSUM evacuation for the v-projection and attention-output goes to
    GpSimd, keeping ACT free for the serial Exp/Ln softplus chain that
    gates phase D; phase D consumes softplus slices in completion order
    via sj-split waves.
  - Causal masking runs in exp-space between Exp and Ln (ln(0+1)=0), so
    diagonal-block masks overlap the off-diagonal Ln chain.
  - DMA: one descriptor only sustains ~50GB/s and costs ~0.6us of issue
    on its engine, so loads are chunked across parallel queues and issued
    from sync/scalar/gpsimd; batch b+1's x/Wv/ang/Wk are prefetched
    during batch b's attention phases at points where the issuing engine
    is idle.
  - y is stored fp16 (2MB/batch) and upcast on host.
"""

import sys
import types
from contextlib import ExitStack

import numpy as np

if "/opt/trn_rl_repo" not in sys.path:
    sys.path.insert(0, "/opt/trn_rl_repo")

B, T, C = 16, 1024, 1024
D = C // 2
NCORES = 8
BPC = B // NCORES          # batches per core
P = 128                    # partitions
NT = T // P                # t tiles
NCH = C // P               # c tiles
ND = D // P                # d tiles (channel pairs)
H = 512                    # matmul free-dim block
PI = float(np.pi)
SCALE = float(C ** -0.5)

_CACHE = {}


def _install_profile_hook():
    """Register the axon NTFF profile hook if the image's antenv lacks it."""
    try:
        import antenv
        from trn_agent_boot.trn_boot import _ntff_profile_via_ctypes
    except Exception:
        return
    if "antenv.axon_hooks" in sys.modules:
        return
    try:
        hook = _ntff_profile_via_ctypes("/opt/axon/libaxon_pjrt.so")
    except Exception:
        return
    mod = types.ModuleType("antenv.axon_hooks")
    mod.get_axon_ntff_profile_hook = lambda: hook
    mod.set_axon_ntff_profile_hook = lambda h: None
    sys.modules["antenv.axon_hooks"] = mod
    antenv.axon_hooks = mod


def _build(has_bias: bool):
    import concourse.bass as bass  # noqa: F401
    import concourse.mybir as mybir
    import concourse.tile as tile
    from concourse import bacc
    from concourse.masks import make_identity

    dt = mybir.dt
    AF = mybir.ActivationFunctionType
    f16 = dt.float16
    f32 = dt.float32
    f8 = dt.float8e4
    DR = mybir.MatmulPerfMode.DoubleRow

    # Keep Exp/Ln in one table set and Sin in trig_and_small so the program
    # avoids mid-phase ACT table switches (a switch costs ~1.3us).
    import concourse.hw_specs as _hw_specs
    if not hasattr(_hw_specs, "_orig_get_activation_tables"):
        _hw_specs._orig_get_activation_tables = _hw_specs.get_activation_tables

        def _filtered_tables(arch):
            tabs = _hw_specs._orig_get_activation_tables(arch)
            for name, fns in tabs.items():
                if name != "natural_log_exp_and_others":
                    fns.discard(AF.Exp)
                    fns.discard(AF.Ln)
                if name != "trig_and_small":
                    fns.discard(AF.Sin)
            return tabs

        _hw_specs.get_activation_tables = _filtered_tables
        bacc.get_activation_tables = _filtered_tables

    nc = bacc.Bacc("TRN2", target_bir_lowering=False, debug=False,
                   num_devices=NCORES)

    xT_d = nc.dram_tensor("xT", [BPC, P, NCH, T], f16, kind="ExternalInput").ap()
    x8_d = nc.dram_tensor("x8", [BPC, P, NCH, T], f8, kind="ExternalInput").ap()
    ang_d = nc.dram_tensor("ang", [BPC, P, NT, D], f16, kind="ExternalInput").ap()
    wk_d = nc.dram_tensor("wk8T", [P, NCH, C], f8, kind="ExternalInput").ap()
    wq_d = nc.dram_tensor("wq8T", [P, NCH, C], f8, kind="ExternalInput").ap()
    wv_d = nc.dram_tensor("wvT", [P, NCH, C], f16, kind="ExternalInput").ap()
    wp_d = nc.dram_tensor("wpT", [P, NCH, C], f16, kind="ExternalInput").ap()
    vmat_d = nc.dram_tensor("vmat", [P, 1408], f16, kind="ExternalInput").ap()
    triu_d = nc.dram_tensor("triu", [P, P], f16, kind="ExternalInput").ap()
    if has_bias:
        bias_d = nc.dram_tensor("biases", [4, 1, C], f16, kind="ExternalInput").ap()
    y_d = nc.dram_tensor("y", [BPC, T, C], f16, kind="ExternalOutput").ap()

    with tile.TileContext(nc) as tc, ExitStack() as ctx:
        const = ctx.enter_context(tc.tile_pool(name="const", bufs=1))
        wv_pool = ctx.enter_context(tc.tile_pool(name="wvpool", bufs=1))
        wpr_pool = ctx.enter_context(tc.tile_pool(name="wprpool", bufs=1))
        w8_pool = ctx.enter_context(tc.tile_pool(name="w8pool", bufs=2))
        x8_pool = ctx.enter_context(tc.tile_pool(name="x8pool", bufs=1))
        xo_pool = ctx.enter_context(tc.tile_pool(name="xopool", bufs=2))
        a_pool = ctx.enter_context(tc.tile_pool(name="apool", bufs=1))
        st_pool = ctx.enter_context(tc.tile_pool(name="stpool", bufs=1))
        ct_pool = ctx.enter_context(tc.tile_pool(name="ctpool", bufs=1))
        sc_pool = ctx.enter_context(tc.tile_pool(name="scpool", bufs=1))
        cc_pool = ctx.enter_context(tc.tile_pool(name="ccpool", bufs=1))
        k_pool = ctx.enter_context(tc.tile_pool(name="kpool", bufs=1))
        q_pool = ctx.enter_context(tc.tile_pool(name="qpool", bufs=1))
        v_pool = ctx.enter_context(tc.tile_pool(name="vpool", bufs=1))
        spw_pool = ctx.enter_context(tc.tile_pool(name="spwpool", bufs=1))
        m_pool = ctx.enter_context(tc.tile_pool(name="mpool", bufs=2))
        g_pool = ctx.enter_context(tc.tile_pool(name="gpool", bufs=2))
        sp_pool = ctx.enter_context(tc.tile_pool(name="sppool", bufs=2))
        y_pool = ctx.enter_context(tc.tile_pool(name="ypool", bufs=2))
        pmm = ctx.enter_context(tc.tile_pool(name="pmm", bufs=8, space="PSUM"))

        # ---- batch-0 input DMA first: engines are idle, land ASAP.
        # HWDGE queues are FIFO with ~2us fixed cost per dma_start, so use
        # few LARGE transfers (small first chunk to unblock the first MM).
        def dma_x(b):
            x_all = xo_pool.tile([P, NCH, T], f16, tag="xo")
            nc.sync.dma_start(x_all[:, 0:2], xT_d[b, :, 0:2])
            nc.sync.dma_start(x_all[:, 2:NCH], xT_d[b, :, 2:NCH])
            return x_all

        def dma_x8(b, engine):
            x8 = x8_pool.tile([P, NCH, T], f8, tag="x8")
            engine.dma_start(x8[:], x8_d[b])
            return x8

        def dma_w8(engine):
            wk8 = w8_pool.tile([P, NCH, C], f8, tag="w8")
            wq8 = w8_pool.tile([P, NCH, C], f8, tag="w8")
            engine.dma_start(wk8[:], wk_d[:])
            engine.dma_start(wq8[:], wq_d[:])
            return wk8, wq8

        def dma_wv(engine):
            wv = wv_pool.tile([P, NCH, C], f16, tag="wv")
            engine.dma_start(wv[:, 0:2], wv_d[:, 0:2])
            engine.dma_start(wv[:, 2:NCH], wv_d[:, 2:NCH])
            return wv

        def dma_ang(b, engine):
            a_all = a_pool.tile([P, NT, D], f16, tag="a")
            engine.dma_start(a_all[:, 4:NT], ang_d[b, :, 4:NT])
            engine.dma_start(a_all[:, 0:4], ang_d[b, :, 0:4])
            return a_all

        def dma_wp(engine):
            w_sb = wpr_pool.tile([P, NCH, C], f16, tag="w")
            engine.dma_start(w_sb[:], wp_d[:])
            return w_sb

        class S:
            pass

        states = [S() for _ in range(BPC)]
        st0 = states[0]
        # ang first: the angle cumsums only need ang+vmat, so they start
        # while the bulkier x/wv streams are still in flight.  Everything
        # the projections need later trails on the gpsimd queue (FIFO).
        st0.x_all = dma_x(0)
        wv = dma_wv(nc.gpsimd)
        gate = const.tile([1, 8], f16)
        gate2 = const.tile([1, 8], f16)
        # cumsums run after the v-pass, so ang/vmat are not start-critical:
        # gate them behind the first x chunk to give the v-pass streams the
        # full HBM bandwidth at kernel start
        nc.scalar.activation(gate2[:], st0.x_all[0:1, 0, 0:8], AF.Copy)
        st0.a_all = dma_ang(0, nc.scalar)

        # consts: vmat[r, u] = 1 iff u <= r + 896 encodes every s>=t step
        # block (its [896:1024] slice is tril).
        vmat = const.tile([P, 1408], f16)
        nc.scalar.dma_start(vmat[:], vmat_d[:])
        triu = const.tile([P, P], f16)
        nc.scalar.dma_start(triu[:], triu_d[:])
        # weights are shared by both batches: load once, gated behind the
        # x16 bulk so the phase-A-critical streams get full HBM bandwidth
        nc.gpsimd.tensor_copy(gate[:], st0.x_all[0:1, NCH - 1, 0:8])
        st0.x8 = dma_x8(0, nc.gpsimd)
        wk8, wq8 = dma_w8(nc.gpsimd)
        wp = dma_wp(nc.gpsimd)
        if has_bias:
            ones_row = const.tile([1, H], f16)
            nc.gpsimd.memset(ones_row[:], 1.0)
            brows = const.tile([1, 4, C], f16)
            for i in range(4):
                nc.scalar.dma_start(brows[:, i], bias_d[i])

        def emit_vproj_and_phaseA(b, st):
            x_all, a_all = st.x_all, st.a_all
            v_all = v_pool.tile([P, NT, C], f16, tag="v")
            st.v_all = v_all

            # phase-A state threaded through the interleaved emission
            sin_t = st_pool.tile([P, NT, D], f16, tag="sin_t")
            cos_t = ct_pool.tile([P, NT, D], f16, tag="cos_t")
            st.sin_t, st.cos_t = sin_t, cos_t

            def vpass(tis):
                pss = {}
                for ti in tis:
                    for ch in range(2):
                        pss[(ti, ch)] = pmm.tile([P, H], f32, tag="pmm",
                                                 name=f"psv{ti}_{ch}")
                for ci in range(NCH):
                    for ti in tis:
                        for ch in range(2):
                            nc.tensor.matmul(
                                pss[(ti, ch)][:],
                                x_all[:, ci, ti * P:(ti + 1) * P],
                                wv[:, ci, ch * H:(ch + 1) * H],
                                start=(ci == 0),
                                stop=(ci == NCH - 1 and not has_bias))
                for ti in tis:
                    for ch in range(2):
                        ps = pss[(ti, ch)]
                        if has_bias:
                            nc.tensor.matmul(ps[:], ones_row[:, :P],
                                             brows[:, 2, ch * H:(ch + 1) * H],
                                             start=False, stop=True)
                        nc.vector.tensor_copy(v_all[:, ti, ch * H:(ch + 1) * H],
                                              ps[:])

            def emit_cumsum_t():
                # t-major reverse cumsum as pure matmuls: psa[tb] =
                # sum_{sb>=tb} V(sb,tb)^T a[sb].  The stationary vmat block
                # depends only on delta = tb-sb, so the inner tb loop keeps
                # one LDWEIGHTS per delta; no serial carry round-trips.
                for half in (1, 0):
                    tbs = range(4 * half, 4 * half + 4)
                    pss = {tb: pmm.tile([P, D], f32, tag="pmm",
                                        name=f"psa{tb}") for tb in tbs}
                    for delta in range(0, -8, -1):
                        for tb in tbs:
                            sb = tb - delta
                            if sb > NT - 1:
                                continue
                            nc.tensor.matmul(
                                pss[tb][:],
                                vmat[:, (7 + delta) * P:(8 + delta) * P],
                                a_all[:, sb],
                                start=(delta == 0), stop=(sb == NT - 1))
                    for tb in tbs:
                        u = sp_pool.tile([P, D], f16, tag="sp")
                        w = sp_pool.tile([P, D], f16, tag="sp")
                        nc.vector.add_range_wrap(u[:], pss[tb][:], 0.0, PI,
                                                 2 * PI)
                        nc.vector.add_range_wrap(w[:], pss[tb][:], PI / 2, PI,
                                                 2 * PI)
                        nc.scalar.activation(sin_t[:, tb], u[:], AF.Sin)
                        nc.scalar.activation(cos_t[:, tb], w[:], AF.Sin)


            def emit_cumsum_c():
                # channel-major angle: reverse-cumsum over tokens computed
                # directly on the PE via vmat step blocks (contraction over
                # the token-tile partitions), then sin/cos per (dj, th).
                sin_c = sc_pool.tile([P, ND, T], f16, tag="sin_c")
                cos_c = cc_pool.tile([P, ND, T], f16, tag="cos_c")
                st.sin_c, st.cos_c = sin_c, cos_c
                for dj in range(ND):
                    psc = [pmm.tile([P, H], f32, tag="pmm", name=f"pcm{dj}0"),
                           pmm.tile([P, H], f32, tag="pmm", name=f"pcm{dj}1")]
                    for sb in range(NT):
                        # the a-tile stationary serves both th halves
                        for th in range(2):
                            if th == 1 and sb < 4:
                                continue  # all-zero step block (s<512<=t)
                            off = (7 - sb) * P + th * H
                            nc.tensor.matmul(psc[th][:],
                                             a_all[:, sb, dj * P:(dj + 1) * P],
                                             vmat[:, off:off + H],
                                             start=(sb == (0 if th == 0 else 4)),
                                             stop=(sb == NT - 1))
                    for th in range(2):
                        hs = slice(th * H, (th + 1) * H)
                        u = sp_pool.tile([P, H], f16, tag="sp")
                        w = sp_pool.tile([P, H], f16, tag="sp")
                        nc.vector.add_range_wrap(u[:], psc[th][:], 0.0, PI,
                                                 2 * PI)
                        nc.vector.add_range_wrap(w[:], psc[th][:], PI / 2, PI,
                                                 2 * PI)
                        nc.scalar.activation(sin_c[:, dj, hs], u[:], AF.Sin)
                        nc.scalar.activation(cos_c[:, dj, hs], w[:], AF.Sin)

            vpass((0, 1))
            vpass((2, 3))
            vpass((4, 5))
            vpass((6, 7))
            emit_cumsum_c()
            emit_cumsum_t()

        def emit_vrot(b, st):
            """Rotate v in place (t-major).  Emitted after the q-projection
            so this long DVE chain overlaps phase C's PE work instead of
            stalling the k-projection's psum evacuations."""
            v_all, sin_t, cos_t = st.v_all, st.sin_t, st.cos_t
            for tj in range(NT // 2):
                tp = slice(2 * tj, 2 * tj + 2)
                z0 = v_all[:, tp, 0:D]
                z1 = v_all[:, tp, D:C]
                cs = cos_t[:, tp]
                sn = sin_t[:, tp]
                ma = m_pool.tile([P, 2, H], f16, tag="ma", bufs=1)
                mb = m_pool.tile([P, 2, H], f16, tag="mb", bufs=1)
                mc = m_pool.tile([P, 2, H], f16, tag="mc", bufs=1)
                md = m_pool.tile([P, 2, H], f16, tag="md", bufs=1)
                nc.vector.tensor_mul(ma[:], z0, cs)
                nc.vector.tensor_mul(mb[:], z1, sn)
                nc.vector.tensor_mul(mc[:], z0, sn)
                nc.vector.tensor_mul(md[:], z1, cs)
                nc.vector.tensor_sub(z0, ma[:], mb[:])
                nc.vector.tensor_add(z1, mc[:], md[:])

        def emit_proj(b, st, w8, out_pool, tag, bias_idx):
            """k/q projection in c-major via fp8 DoubleRow (weights carry a
            x32 prescale; folded out at the Exp).  Rotation output f16."""
            x8 = st.x8
            cos_c, sin_c = st.cos_c, st.sin_c
            out16 = out_pool.tile([P, NCH, T], f16, tag=tag)
            for p in range(ND):
                # psum lands in the final tile; the rotation rewrites it in
                # place (all four products are read into scratch first), so
                # there is no z staging tile and no WAR chain between the
                # ACT evacuations of consecutive p groups.
                for zi, co in ((0, p), (1, p + 4)):
                    pst = [pmm.tile([P, H], f32, tag="pmm", name=f"pj{zi}0"),
                           pmm.tile([P, H], f32, tag="pmm", name=f"pj{zi}1")]
                    for cj in range(NCH // 2):
                        for th in range(2):
                            # both th halves share one LDWEIGHTS
                            nc.tensor.matmul(pst[th][:],
                                             w8[:, 2 * cj:2 * cj + 2,
                                                co * P:(co + 1) * P],
                                             x8[:, 2 * cj:2 * cj + 2,
                                                th * H:(th + 1) * H],
                                             start=(cj == 0),
                                             stop=(cj == NCH // 2 - 1
                                                   and not has_bias),
                                             perf_mode=DR)
                    for th in range(2):
                        if has_bias:
                            nc.tensor.matmul(pst[th][:],
                                             brows[:, bias_idx, co * P:(co + 1) * P],
                                             ones_row[:], start=False, stop=True)
                        nc.scalar.activation(out16[:, co, th * H:(th + 1) * H],
                                             pst[th][:], AF.Copy)
                cs = cos_c[:, p, :]
                sn = sin_c[:, p, :]
                z0 = out16[:, p, :]
                z1 = out16[:, p + 4, :]
                ma = m_pool.tile([P, T], f16, tag="ma", bufs=1)
                mb = m_pool.tile([P, T], f16, tag="mb", bufs=1)
                mc = m_pool.tile([P, T], f16, tag="mc", bufs=1)
                md = g_pool.tile([P, T], f16, tag="ga", bufs=1)
                nc.gpsimd.tensor_mul(md[:], z1, cs)
                nc.vector.tensor_mul(ma[:], z0, cs)
                nc.vector.tensor_mul(mb[:], z1, sn)
                nc.vector.tensor_mul(mc[:], z0, sn)
                nc.vector.tensor_sub(z0, ma[:], mb[:])
                nc.vector.tensor_add(z1, mc[:], md[:])
            return out16

        def emit_phaseC(b, st):
            """wei^T = softplus(q.k / sqrt(C)): f16 logits, Exp,
            exp-space causal mask, Ln.  The x32 weight prescale on both
            k and q is folded out via the Exp scale (1/1024)."""
            k16, q16 = st.k16, st.q16
            spw = spw_pool.tile([P, 12, H], f16, tag="spw")
            st.spw = spw
            # one pass over si: the q-block stationary serves both
            # th-halves (one LDWEIGHTS per ci).  Exp-space diagonal mask +
            # Ln run per-slice so early slices are ready for phase D fast.
            for si in range(NT):
                has0 = si < 4
                off0 = si * P
                off1 = max(0, si * P - H)
                ps0 = (pmm.tile([P, H], f32, tag="pmm", name="pc0")
                       if has0 else None)
                ps1 = pmm.tile([P, H], f32, tag="pmm", name="pc1")
                for j, ci in enumerate((0, 4, 2, 6, 1, 5, 3, 7)):
                    if has0:
                        nc.tensor.matmul(ps0[:, 0:H - off0],
                                         q16[:, ci, si * P:(si + 1) * P],
                                         k16[:, ci, off0:H],
                                         start=(j == 0), stop=(j == NCH - 1))
                    nc.tensor.matmul(ps1[:, 0:H - off1],
                                     q16[:, ci, si * P:(si + 1) * P],
                                     k16[:, ci, H + off1:T],
                                     start=(j == 0), stop=(j == NCH - 1))
                if has0:
                    nc.scalar.activation(spw[:, si, off0:H],
                                         ps0[:, 0:H - off0], AF.Exp,
                                         scale=SCALE / 1024.0)
                nc.scalar.activation(spw[:, 4 + si, off1:H],
                                     ps1[:, 0:H - off1], AF.Exp,
                                     scale=SCALE / 1024.0)
                if has0:
                    nc.vector.tensor_mul(spw[:, si, off0:off0 + P],
                                         spw[:, si, off0:off0 + P], triu[:])
                    nc.scalar.activation(spw[:, si, off0:H],
                                         spw[:, si, off0:H], AF.Ln, bias=1.0)
                else:
                    nc.vector.tensor_mul(spw[:, 4 + si, off1:off1 + P],
                                         spw[:, 4 + si, off1:off1 + P],
                                         triu[:])
                nc.scalar.activation(spw[:, 4 + si, off1:H],
                                     spw[:, 4 + si, off1:H], AF.Ln, bias=1.0)

        def emit_phaseD(b, st):
            """out^T = v.T @ wei^T, inverse-rotated -> ro.  sj-split waves:
            each psum group first accumulates the early softplus slices so
            the tail of the ACT chain is only needed late."""
            v_all, spw = st.v_all, st.spw
            cos_c, sin_c = st.cos_c, st.sin_c
            ro = xo_pool.tile([P, NCH, T], f16, tag="xo")
            st.ro = ro

            def group(th, pps, waves):
                smax = 4 * th + 3
                pss = {}
                for pp in pps:
                    pss[(pp, 0)] = pmm.tile([P, H], f32, tag="pmm",
                                            name=f"psd{pp}_0")
                    pss[(pp, 1)] = pmm.tile([P, H], f32, tag="pmm",
                                            name=f"psd{pp}_1")
                for wave in waves:
                    for pp in pps:
                        for zi, pq in ((0, pp), (1, pp + 4)):
                            ps = pss[(pp, zi)]
                            for sj in wave:
                                off = max(0, sj * P - th * H)
                                nc.tensor.matmul(
                                    ps[:, off:H],
                                    v_all[:, sj, pq * P:(pq + 1) * P],
                                    spw[:, 4 * th + sj, off:H],
                                    start=(sj == 0), stop=(sj == smax))
                for pp in pps:
                    hs = slice(th * H, (th + 1) * H)
                    cs = cos_c[:, pp, hs]
                    sn = sin_c[:, pp, hs]
                    # evacuate psum straight into ro, then rotate in place;
                    # the four products are read into scratch before the
                    # overwrites, split gpsimd/DVE so neither gates phase D
                    oz0 = ro[:, pp, hs]
                    oz1 = ro[:, pp + 4, hs]
                    # ACT is idle in the D window: let it evacuate psum
                    nc.scalar.activation(oz0, pss[(pp, 0)][:], AF.Copy)
                    nc.scalar.activation(oz1, pss[(pp, 1)][:], AF.Copy)
                    ga = g_pool.tile([P, H], f16, tag="ga", bufs=1)
                    nc.gpsimd.tensor_mul(ga[:], oz0, cs)
                    ma = m_pool.tile([P, H], f16, tag="ma", bufs=1)
                    mb = m_pool.tile([P, H], f16, tag="mb", bufs=1)
                    mc = m_pool.tile([P, H], f16, tag="mc", bufs=1)
                    nc.vector.tensor_mul(mc[:], oz1, sn)
                    nc.vector.tensor_mul(ma[:], oz0, sn)
                    nc.vector.tensor_mul(mb[:], oz1, cs)
                    nc.vector.tensor_add(oz0, ga[:], mc[:])
                    nc.vector.tensor_sub(oz1, mb[:], ma[:])

            group(0, (0, 1, 2), ((0, 1), (2, 3)))
            group(0, (3,), ((0, 1), (2, 3)))
            group(1, (0, 1, 2), ((0, 1, 2, 3), (4, 5, 6, 7)))
            group(1, (3,), ((0, 1, 2, 3), (4, 5, 6, 7)))

        def emit_phaseE(b, st, wp):
            ro = st.ro
            ci_order = [0, 1, 2, 4, 5, 6, 3, 7]
            for ti in range(NT):
                pse = [pmm.tile([P, H], f32, tag="pmm", name="pe0"),
                       pmm.tile([P, H], f32, tag="pmm", name="pe1")]
                for j, ci in enumerate(ci_order):
                    for ch in range(2):
                        # both ch halves share one LDWEIGHTS
                        nc.tensor.matmul(pse[ch][:],
                                         ro[:, ci, ti * P:(ti + 1) * P],
                                         wp[:, ci, ch * H:(ch + 1) * H],
                                         start=(j == 0),
                                         stop=(j == NCH - 1 and not has_bias))
                yt = y_pool.tile([P, C], f16, tag="y")
                for ch in range(2):
                    if has_bias:
                        nc.tensor.matmul(pse[ch][:], ones_row[:, :P],
                                         brows[:, 3, ch * H:(ch + 1) * H],
                                         start=False, stop=True)
                    nc.scalar.activation(yt[:, ch * H:(ch + 1) * H],
                                         pse[ch][:], AF.Copy)
                # one big store per ti, alternating queues so the store
                # stream drains inside phase E instead of tailing after it
                eng = nc.sync if ti % 2 == 0 else nc.gpsimd
                eng.dma_start(y_d[b, ti * P:(ti + 1) * P, :], yt[:])

        # ================= schedule =================
        for b in range(BPC):
            st = states[b]
            emit_vproj_and_phaseA(b, st)
            # prefetch next batch's x/x8/ang while PE chews on k-proj;
            # sync and gpsimd queues are otherwise idle here
            if b + 1 < BPC:
                nxt = states[b + 1]
                nxt.x_all = dma_x(b + 1)
                nxt.x8 = dma_x8(b + 1, nc.sync)
            st.k16 = emit_proj(b, st, wk8, k_pool, "k", 0)
            if b + 1 < BPC:
                nxt.a_all = dma_ang(b + 1, nc.gpsimd)
            st.q16 = emit_proj(b, st, wq8, q_pool, "q", 1)
            emit_vrot(b, st)
            emit_phaseC(b, st)
            emit_phaseD(b, st)
            emit_phaseE(b, st, wp)

    nc.compile()
    return nc


def _get_program(has_bias: bool):
    key = ("prog3", has_bias)
    if key not in _CACHE:
        _CACHE[key] = _build(has_bias)
    return _CACHE[key]


def _prep_host(x, idx, Wk, Wq, Wv, Wp, ang_emb, biases):
    import ml_dtypes
    e4 = ml_dtypes.float8_e4m3
    perm = np.concatenate([np.arange(0, C, 2), np.arange(1, C, 2)])
    # x: [B, T, C] -> per batch [P, NCH, T] (partition-major chunks of x^T)
    xT = np.transpose(np.asarray(x, np.float32), (0, 2, 1))      # [B, C, T]
    xTt = xT.reshape(B, NCH, P, T)
    xTt = np.ascontiguousarray(np.transpose(xTt, (0, 2, 1, 3)))
    xT16 = xTt.astype(np.float16).reshape(NCORES, BPC, P, NCH, T)
    xT8 = xTt.astype(e4).reshape(NCORES, BPC, P, NCH, T)
    idx = np.asarray(idx).astype(np.int64)
    ang = np.asarray(ang_emb, np.float32)[idx]                   # [B, T, D]
    ang16 = ang.astype(np.float16).reshape(B, NT, P, D)
    ang16 = np.ascontiguousarray(np.transpose(ang16, (0, 2, 1, 3)))
    ang16 = ang16.reshape(NCORES, BPC, P, NT, D)

    def wtile(m, dtype=np.float16):
        w = np.ascontiguousarray(m).astype(dtype).reshape(NCH, P, C)
        return np.ascontiguousarray(np.transpose(w, (1, 0, 2)))

    # k/q weights carry x32 so fp8e4 values sit in the normal range;
    # folded out by the Exp scale (1/1024) in phase C.
    wk8T = wtile(np.asarray(Wk, np.float32)[perm].T * 32.0, e4)
    wq8T = wtile(np.asarray(Wq, np.float32)[perm].T * 32.0, e4)
    wvT = wtile(np.asarray(Wv, np.float32)[perm].T)
    wpT = wtile(np.asarray(Wp, np.float32)[:, perm].T)

    vmat = (np.arange(1408)[None, :] <= np.arange(P)[:, None] + 896)
    vmat = vmat.astype(np.float16)
    triu = np.triu(np.ones((P, P), np.float16))

    consts = dict(wk8T=wk8T, wq8T=wq8T, wvT=wvT, wpT=wpT, vmat=vmat, triu=triu)
    bk, bq, bv, bp = (np.asarray(b_, np.float32) for b_ in biases)
    has_bias = any(np.any(b_ != 0) for b_ in (bk, bq, bv, bp))
    if has_bias:
        brows = np.stack([bk[perm] * 32.0, bq[perm] * 32.0, bv[perm],
                          bp]).astype(np.float16)
        consts["biases"] = brows.reshape(4, 1, C)
    return xT16, xT8, ang16, consts, has_bias


def run_on_device(x, idx, Wk, Wq, Wv, Wp, ang_emb, biases, trace=False):
    _install_profile_hook()
    import concourse.bass_utils as bass_utils
    bass_utils.upload_artifacts = lambda tmpdir: "local://" + tmpdir
    from concourse.bass_utils import run_bass_kernel_spmd

    xT16, xT8, ang16, consts, has_bias = _prep_host(x, idx, Wk, Wq, Wv, Wp,
                                                    ang_emb, biases)
    nc = _get_program(has_bias)
    in_maps = []
    for c in range(NCORES):
        m = {"xT": xT16[c], "x8": xT8[c], "ang": ang16[c]}
        m.update(consts)
        in_maps.append(m)
    res = run_bass_kernel_spmd(nc, in_maps, list(range(NCORES)), trace=trace)
    y = np.empty((B, T, C), np.float32)
    for c in range(NCORES):
        y[c * BPC:(c + 1) * BPC] = res.results[c]["y"].astype(np.float32)
    return y, res


def kernel(x, idx, Wk, bk, Wq, bq, Wv, bv, Wp, bp, ang_emb):
    y, _ = run_on_device(x, idx, Wk, Wq, Wv, Wp, ang_emb, (bk, bq, bv, bp))
    return y



# revision 31
# speedup vs baseline: 1.2581x; 1.2581x over previous
"""Trainium2 Bass kernel for CumsumAttention (v3).

Full-input contract: kernel(**inputs) takes the complete (unsharded) inputs
and returns the full [B, T, C] float32 output. Internally the work is
data-parallel over the batch dimension across 8 NeuronCores (2 batches per
core); each core runs the same Bass/Tile program on its own batch shard.

Math (per batch, reference semantics):
  k = x @ Wk.T ; q = x @ Wq.T ; v = x @ Wv.T   (biases all zero here)
  angle[t] = sum_{s>=t} ang_emb[idx[s]]        (reverse cumsum over T)
  rot/inv_rot: per channel-pair rotation by angle
  wei  = softplus((rot(k) @ rot(q).T) / sqrt(C)) masked causally (s <= t)
  out  = inv_rot(wei @ rot(v)) @ Wp.T

Engine plan (vs the 337.5us fp16 baseline):
  - Logits matmul in fp8e4 DoubleRow (2 K-tiles per instruction, 2x MAC
    rate); rot(k)/rot(q) written to fp8 by the DVE rotation.  Measured
    rel err ~1.3e-2 < 2e-2.
  - v-projection accumulates ci-outer across PSUM-resident groups so
    compute starts on the first x/Wv chunk; the angle-cumsum PE chain is
    interleaved one tile per chunk so its carry round-trips hide.
  - PSUM evacuation for the v-projection and attention-output goes to
    GpSimd, keeping ACT free for the serial Exp/Ln softplus chain that
    gates phase D; phase D consumes softplus slices in completion order
    via sj-split waves.
  - Causal masking runs in exp-space between Exp and Ln (ln(0+1)=0), so
    diagonal-block masks overlap the off-diagonal Ln chain.
  - DMA: one descriptor only sustains ~50GB/s and costs ~0.6us of issue
    on its engine, so loads are chunked across parallel queues and issued
    from sync/scalar/gpsimd; batch b+1's x/Wv/ang/Wk are prefetched
    during batch b's attention phases at points where the issuing engine
    is idle.
  - y is stored fp16 (2MB/batch) and upcast on host.
"""

import sys
import types
from contextlib import ExitStack

import numpy as np

if "/opt/trn_rl_repo" not in sys.path:
    sys.path.insert(0, "/opt/trn_rl_repo")

B, T, C = 16, 1024, 1024
D = C // 2
NCORES = 8
BPC = B // NCORES          # batches per core
P = 128                    # partitions
NT = T // P                # t tiles
NCH = C // P               # c tiles
ND = D // P                # d tiles (channel pairs)
H = 512                    # matmul free-dim block
PI = float(np.pi)
SCALE = float(C ** -0.5)

_CACHE = {}


def _install_profile_hook():
    """Register the axon NTFF profile hook if the image's antenv lacks it."""
    try:
        import antenv
        from trn_agent_boot.trn_boot import _ntff_profile_via_ctypes
    except Exception:
        return
    if "antenv.axon_hooks" in sys.modules:
        return
    try:
        hook = _ntff_profile_via_ctypes("/opt/axon/libaxon_pjrt.so")
    except Exception:
        return
    mod = types.ModuleType("antenv.axon_hooks")
    mod.get_axon_ntff_profile_hook = lambda: hook
    mod.set_axon_ntff_profile_hook = lambda h: None
    sys.modules["antenv.axon_hooks"] = mod
    antenv.axon_hooks = mod


def _build(has_bias: bool):
    import concourse.bass as bass  # noqa: F401
    import concourse.mybir as mybir
    import concourse.tile as tile
    from concourse import bacc
    from concourse.masks import make_identity

    dt = mybir.dt
    AF = mybir.ActivationFunctionType
    f16 = dt.float16
    f32 = dt.float32
    f8 = dt.float8e4
    DR = mybir.MatmulPerfMode.DoubleRow

    # Keep Exp/Ln in one table set and Sin in trig_and_small so the program
    # avoids mid-phase ACT table switches (a switch costs ~1.3us).
    import concourse.hw_specs as _hw_specs
    if not hasattr(_hw_specs, "_orig_get_activation_tables"):
        _hw_specs._orig_get_activation_tables = _hw_specs.get_activation_tables

        def _filtered_tables(arch):
            tabs = _hw_specs._orig_get_activation_tables(arch)
            for name, fns in tabs.items():
                if name != "natural_log_exp_and_others":
                    fns.discard(AF.Exp)
                    fns.discard(AF.Ln)
                if name != "trig_and_small":
                    fns.discard(AF.Sin)
            return tabs

        _hw_specs.get_activation_tables = _filtered_tables
        bacc.get_activation_tables = _filtered_tables

    nc = bacc.Bacc("TRN2", target_bir_lowering=False, debug=False,
                   num_devices=NCORES)

    xT_d = nc.dram_tensor("xT", [BPC, P, NCH, T], f16, kind="ExternalInput").ap()
    x8_d = nc.dram_tensor("x8", [BPC, P, NCH, T], f8, kind="ExternalInput").ap()
    ang_d = nc.dram_tensor("ang", [BPC, P, NT, D], f16, kind="ExternalInput").ap()
    wk_d = nc.dram_tensor("wk8T", [P, NCH, C], f8, kind="ExternalInput").ap()
    wq_d = nc.dram_tensor("wq8T", [P, NCH, C], f8, kind="ExternalInput").ap()
    wv_d = nc.dram_tensor("wvT", [P, NCH, C], f16, kind="ExternalInput").ap()
    wp_d = nc.dram_tensor("wpT", [P, NCH, C], f16, kind="ExternalInput").ap()
    vmat_d = nc.dram_tensor("vmat", [P, 1408], f16, kind="ExternalInput").ap()
    triu_d = nc.dram_tensor("triu", [P, P], f16, kind="ExternalInput").ap()
    if has_bias:
        bias_d = nc.dram_tensor("biases", [4, 1, C], f16, kind="ExternalInput").ap()
    y_d = nc.dram_tensor("y", [BPC, T, C], f16, kind="ExternalOutput").ap()

    with tile.TileContext(nc) as tc, ExitStack() as ctx:
        const = ctx.enter_context(tc.tile_pool(name="const", bufs=1))
        wv_pool = ctx.enter_context(tc.tile_pool(name="wvpool", bufs=1))
        wpr_pool = ctx.enter_context(tc.tile_pool(name="wprpool", bufs=1))
        w8_pool = ctx.enter_context(tc.tile_pool(name="w8pool", bufs=2))
        x8_pool = ctx.enter_context(tc.tile_pool(name="x8pool", bufs=1))
        xo_pool = ctx.enter_context(tc.tile_pool(name="xopool", bufs=2))
        a_pool = ctx.enter_context(tc.tile_pool(name="apool", bufs=1))
        st_pool = ctx.enter_context(tc.tile_pool(name="stpool", bufs=1))
        ct_pool = ctx.enter_context(tc.tile_pool(name="ctpool", bufs=1))
        sc_pool = ctx.enter_context(tc.tile_pool(name="scpool", bufs=1))
        cc_pool = ctx.enter_context(tc.tile_pool(name="ccpool", bufs=1))
        k_pool = ctx.enter_context(tc.tile_pool(name="kpool", bufs=1))
        q_pool = ctx.enter_context(tc.tile_pool(name="qpool", bufs=1))
        v_pool = ctx.enter_context(tc.tile_pool(name="vpool", bufs=1))
        spw_pool = ctx.enter_context(tc.tile_pool(name="spwpool", bufs=1))
        m_pool = ctx.enter_context(tc.tile_pool(name="mpool", bufs=2))
        g_pool = ctx.enter_context(tc.tile_pool(name="gpool", bufs=2))
        sp_pool = ctx.enter_context(tc.tile_pool(name="sppool", bufs=2))
        y_pool = ctx.enter_context(tc.tile_pool(name="ypool", bufs=2))
        pmm = ctx.enter_context(tc.tile_pool(name="pmm", bufs=8, space="PSUM"))

        # ---- batch-0 input DMA first: engines are idle, land ASAP.
        # HWDGE queues are FIFO with ~2us fixed cost per dma_start, so use
        # few LARGE transfers (small first chunk to unblock the first MM).
        def dma_x(b):
            x_all = xo_pool.tile([P, NCH, T], f16, tag="xo")
            nc.sync.dma_start(x_all[:, 0:2], xT_d[b, :, 0:2])
            nc.sync.dma_start(x_all[:, 2:NCH], xT_d[b, :, 2:NCH])
            return x_all

        def dma_x8(b, engine):
            x8 = x8_pool.tile([P, NCH, T], f8, tag="x8")
            engine.dma_start(x8[:], x8_d[b])
            return x8

        def dma_w8(engine):
            wk8 = w8_pool.tile([P, NCH, C], f8, tag="w8")
            wq8 = w8_pool.tile([P, NCH, C], f8, tag="w8")
            engine.dma_start(wk8[:], wk_d[:])
            engine.dma_start(wq8[:], wq_d[:])
            return wk8, wq8

        def dma_wv(engine):
            wv = wv_pool.tile([P, NCH, C], f16, tag="wv")
            engine.dma_start(wv[:, 0:2], wv_d[:, 0:2])
            engine.dma_start(wv[:, 2:NCH], wv_d[:, 2:NCH])
            return wv

        def dma_ang(b, engine):
            a_all = a_pool.tile([P, NT, D], f16, tag="a")
            engine.dma_start(a_all[:, 4:NT], ang_d[b, :, 4:NT])
            engine.dma_start(a_all[:, 0:4], ang_d[b, :, 0:4])
            return a_all

        def dma_wp(engine):
            w_sb = wpr_pool.tile([P, NCH, C], f16, tag="w")
            engine.dma_start(w_sb[:], wp_d[:])
            return w_sb

        class S:
            pass

        states = [S() for _ in range(BPC)]
        st0 = states[0]
        # ang first: the angle cumsums only need ang+vmat, so they start
        # while the bulkier x/wv streams are still in flight.  Everything
        # the projections need later trails on the gpsimd queue (FIFO).
        st0.x_all = dma_x(0)
        wv = dma_wv(nc.gpsimd)
        gate = const.tile([1, 8], f16)
        gate2 = const.tile([1, 8], f16)
        # cumsums run after the v-pass, so ang/vmat are not start-critical:
        # gate them behind the first x chunk to give the v-pass streams the
        # full HBM bandwidth at kernel start
        nc.scalar.activation(gate2[:], st0.x_all[0:1, 0, 0:8], AF.Copy)
        st0.a_all = dma_ang(0, nc.scalar)

        # consts: vmat[r, u] = 1 iff u <= r + 896 encodes every s>=t step
        # block (its [896:1024] slice is tril).
        vmat = const.tile([P, 1408], f16)
        nc.scalar.dma_start(vmat[:], vmat_d[:])
        triu = const.tile([P, P], f16)
        nc.scalar.dma_start(triu[:], triu_d[:])
        # weights are shared by both batches: load once, gated behind the
        # x16 bulk so the phase-A-critical streams get full HBM bandwidth
        nc.gpsimd.tensor_copy(gate[:], st0.x_all[0:1, NCH - 1, 0:8])
        st0.x8 = dma_x8(0, nc.gpsimd)
        wk8, wq8 = dma_w8(nc.gpsimd)
        wp = dma_wp(nc.gpsimd)
        if has_bias:
            ones_row = const.tile([1, H], f16)
            nc.gpsimd.memset(ones_row[:], 1.0)
            brows = const.tile([1, 4, C], f16)
            for i in range(4):
                nc.scalar.dma_start(brows[:, i], bias_d[i])

        def emit_vproj_and_phaseA(b, st):
            x_all, a_all = st.x_all, st.a_all
            v_all = v_pool.tile([P, NT, C], f16, tag="v")
            st.v_all = v_all

            # phase-A state threaded through the interleaved emission
            sin_t = st_pool.tile([P, NT, D], f16, tag="sin_t")
            cos_t = ct_pool.tile([P, NT, D], f16, tag="cos_t")
            st.sin_t, st.cos_t = sin_t, cos_t

            def vpass(tis):
                pss = {}
                for ti in tis:
                    for ch in range(2):
                        pss[(ti, ch)] = pmm.tile([P, H], f32, tag="pmm",
                                                 name=f"psv{ti}_{ch}")
                for ci in range(NCH):
                    for ti in tis:
                        for ch in range(2):
                            nc.tensor.matmul(
                                pss[(ti, ch)][:],
                                x_all[:, ci, ti * P:(ti + 1) * P],
                                wv[:, ci, ch * H:(ch + 1) * H],
                                start=(ci == 0),
                                stop=(ci == NCH - 1 and not has_bias))
                for ti in tis:
                    for ch in range(2):
                        ps = pss[(ti, ch)]
                        if has_bias:
                            nc.tensor.matmul(ps[:], ones_row[:, :P],
                                             brows[:, 2, ch * H:(ch + 1) * H],
                                             start=False, stop=True)
                        nc.vector.tensor_copy(v_all[:, ti, ch * H:(ch + 1) * H],
                                              ps[:])

            def emit_cumsum_t():
                # t-major reverse cumsum as pure matmuls: psa[tb] =
                # sum_{sb>=tb} V(sb,tb)^T a[sb].  The stationary vmat block
                # depends only on delta = tb-sb, so the inner tb loop keeps
                # one LDWEIGHTS per delta; no serial carry round-trips.
                for half in (1, 0):
                    tbs = range(4 * half, 4 * half + 4)
                    pss = {tb: pmm.tile([P, D], f32, tag="pmm",
                                        name=f"psa{tb}") for tb in tbs}
                    for delta in range(0, -8, -1):
                        for tb in tbs:
                            sb = tb - delta
                            if sb > NT - 1:
                                continue
                            nc.tensor.matmul(
                                pss[tb][:],
                                vmat[:, (7 + delta) * P:(8 + delta) * P],
                                a_all[:, sb],
                                start=(delta == 0), stop=(sb == NT - 1))
                    for tb in tbs:
                        u = sp_pool.tile([P, D], f16, tag="sp")
                        w = sp_pool.tile([P, D], f16, tag="sp")
                        nc.vector.add_range_wrap(u[:], pss[tb][:], 0.0, PI,
                                                 2 * PI)
                        nc.vector.add_range_wrap(w[:], pss[tb][:], PI / 2, PI,
                                                 2 * PI)
                        nc.scalar.activation(sin_t[:, tb], u[:], AF.Sin)
                        nc.scalar.activation(cos_t[:, tb], w[:], AF.Sin)


            def emit_cumsum_c():
                # channel-major angle: reverse-cumsum over tokens computed
                # directly on the PE via vmat step blocks (contraction over
                # the token-tile partitions), then sin/cos per (dj, th).
                sin_c = sc_pool.tile([P, ND, T], f16, tag="sin_c")
                cos_c = cc_pool.tile([P, ND, T], f16, tag="cos_c")
                st.sin_c, st.cos_c = sin_c, cos_c
                for dj in range(ND):
                    psc = [pmm.tile([P, H], f32, tag="pmm", name=f"pcm{dj}0"),
                           pmm.tile([P, H], f32, tag="pmm", name=f"pcm{dj}1")]
                    for sb in range(NT):
                        # the a-tile stationary serves both th halves
                        for th in range(2):
                            if th == 1 and sb < 4:
                                continue  # all-zero step block (s<512<=t)
                            off = (7 - sb) * P + th * H
                            nc.tensor.matmul(psc[th][:],
                                             a_all[:, sb, dj * P:(dj + 1) * P],
                                             vmat[:, off:off + H],
                                             start=(sb == (0 if th == 0 else 4)),
                                             stop=(sb == NT - 1))
                    for th in range(2):
                        hs = slice(th * H, (th + 1) * H)
                        u = sp_pool.tile([P, H], f16, tag="sp")
                        w = sp_pool.tile([P, H], f16, tag="sp")
                        nc.vector.add_range_wrap(u[:], psc[th][:], 0.0, PI,
                                                 2 * PI)
                        nc.vector.add_range_wrap(w[:], psc[th][:], PI / 2, PI,
                                                 2 * PI)
                        nc.scalar.activation(sin_c[:, dj, hs], u[:], AF.Sin)
                        nc.scalar.activation(cos_c[:, dj, hs], w[:], AF.Sin)

            vpass((0, 1))
            vpass((2, 3))
            vpass((4, 5))
            vpass((6, 7))
            emit_cumsum_c()
            emit_cumsum_t()

        def emit_vrot(b, st):
            """Rotate v in place (t-major).  Emitted after the q-projection
            so this long DVE chain overlaps phase C's PE work instead of
            stalling the k-projection's psum evacuations."""
            v_all, sin_t, cos_t = st.v_all, st.sin_t, st.cos_t
            for tj in range(NT // 2):
                tp = slice(2 * tj, 2 * tj + 2)
                z0 = v_all[:, tp, 0:D]
                z1 = v_all[:, tp, D:C]
                cs = cos_t[:, tp]
                sn = sin_t[:, tp]
                ma = m_pool.tile([P, 2, H], f16, tag="ma", bufs=1)
                mb = m_pool.tile([P, 2, H], f16, tag="mb", bufs=1)
                mc = m_pool.tile([P, 2, H], f16, tag="mc", bufs=1)
                md = m_pool.tile([P, 2, H], f16, tag="md", bufs=1)
                nc.vector.tensor_mul(ma[:], z0, cs)
                nc.vector.tensor_mul(mb[:], z1, sn)
                nc.vector.tensor_mul(mc[:], z0, sn)
                nc.vector.tensor_mul(md[:], z1, cs)
                nc.vector.tensor_sub(z0, ma[:], mb[:])
                nc.vector.tensor_add(z1, mc[:], md[:])

        def emit_proj(b, st, w8, out_pool, tag, bias_idx):
            """k/q projection in c-major via fp8 DoubleRow (weights carry a
            x32 prescale; folded out at the Exp).  Rotation output f16."""
            x8 = st.x8
            cos_c, sin_c = st.cos_c, st.sin_c
            out16 = out_pool.tile([P, NCH, T], f16, tag=tag)
            for p in range(ND):
                # psum lands in the final tile; the rotation rewrites it in
                # place (all four products are read into scratch first), so
                # there is no z staging tile and no WAR chain between the
                # ACT evacuations of consecutive p groups.
                for zi, co in ((0, p), (1, p + 4)):
                    pst = [pmm.tile([P, H], f32, tag="pmm", name=f"pj{zi}0"),
                           pmm.tile([P, H], f32, tag="pmm", name=f"pj{zi}1")]
                    for cj in range(NCH // 2):
                        for th in range(2):
                            # both th halves share one LDWEIGHTS
                            nc.tensor.matmul(pst[th][:],
                                             w8[:, 2 * cj:2 * cj + 2,
                                                co * P:(co + 1) * P],
                                             x8[:, 2 * cj:2 * cj + 2,
                                                th * H:(th + 1) * H],
                                             start=(cj == 0),
                                             stop=(cj == NCH // 2 - 1
                                                   and not has_bias),
                                             perf_mode=DR)
                    for th in range(2):
                        if has_bias:
                            nc.tensor.matmul(pst[th][:],
                                             brows[:, bias_idx, co * P:(co + 1) * P],
                                             ones_row[:], start=False, stop=True)
                        nc.scalar.activation(out16[:, co, th * H:(th + 1) * H],
                                             pst[th][:], AF.Copy)
                cs = cos_c[:, p, :]
                sn = sin_c[:, p, :]
                z0 = out16[:, p, :]
                z1 = out16[:, p + 4, :]
                ma = m_pool.tile([P, T], f16, tag="ma", bufs=1)
                mb = m_pool.tile([P, T], f16, tag="mb", bufs=1)
                mc = m_pool.tile([P, T], f16, tag="mc", bufs=1)
                md = m_pool.tile([P, T], f16, tag="md", bufs=1)
                nc.vector.tensor_mul(ma[:], z0, cs)
                nc.vector.tensor_mul(mb[:], z1, sn)
                nc.vector.tensor_mul(mc[:], z0, sn)
                nc.vector.tensor_mul(md[:], z1, cs)
                nc.vector.tensor_sub(z0, ma[:], mb[:])
                nc.vector.tensor_add(z1, mc[:], md[:])
            return out16

        def emit_phaseC(b, st):
            """wei^T = softplus(q.k / sqrt(C)): f16 logits, Exp,
            exp-space causal mask, Ln.  The x32 weight prescale on both
            k and q is folded out via the Exp scale (1/1024)."""
            k16, q16 = st.k16, st.q16
            spw = spw_pool.tile([P, 12, H], f16, tag="spw")
            st.spw = spw
            # one pass over si: the q-block stationary serves both
            # th-halves (one LDWEIGHTS per ci).  Exp-space diagonal mask +
            # Ln run per-slice so early slices are ready for phase D fast.
            for si in range(NT):
                has0 = si < 4
                off0 = si * P
                off1 = max(0, si * P - H)
                ps0 = (pmm.tile([P, H], f32, tag="pmm", name="pc0")
                       if has0 else None)
                ps1 = pmm.tile([P, H], f32, tag="pmm", name="pc1")
                for j, ci in enumerate((0, 4, 2, 6, 1, 5, 3, 7)):
                    if has0:
                        nc.tensor.matmul(ps0[:, 0:H - off0],
                                         q16[:, ci, si * P:(si + 1) * P],
                                         k16[:, ci, off0:H],
                                         start=(j == 0), stop=(j == NCH - 1))
                    nc.tensor.matmul(ps1[:, 0:H - off1],
                                     q16[:, ci, si * P:(si + 1) * P],
                                     k16[:, ci, H + off1:T],
                                     start=(j == 0), stop=(j == NCH - 1))
                if has0:
                    nc.scalar.activation(spw[:, si, off0:H],
                                         ps0[:, 0:H - off0], AF.Exp,
                                         scale=SCALE / 1024.0)
                nc.scalar.activation(spw[:, 4 + si, off1:H],
                                     ps1[:, 0:H - off1], AF.Exp,
                                     scale=SCALE / 1024.0)
                if has0:
                    nc.vector.tensor_mul(spw[:, si, off0:off0 + P],
                                         spw[:, si, off0:off0 + P], triu[:])
                    nc.scalar.activation(spw[:, si, off0:H],
                                         spw[:, si, off0:H], AF.Ln, bias=1.0)
                else:
                    nc.vector.tensor_mul(spw[:, 4 + si, off1:off1 + P],
                                         spw[:, 4 + si, off1:off1 + P],
                                         triu[:])
                nc.scalar.activation(spw[:, 4 + si, off1:H],
                                     spw[:, 4 + si, off1:H], AF.Ln, bias=1.0)

        def emit_phaseD(b, st):
            """out^T = v.T @ wei^T, inverse-rotated -> ro.  sj-split waves:
            each psum group first accumulates the early softplus slices so
            the tail of the ACT chain is only needed late."""
            v_all, spw = st.v_all, st.spw
            cos_c, sin_c = st.cos_c, st.sin_c
            ro = xo_pool.tile([P, NCH, T], f16, tag="xo")
            st.ro = ro

            def group(th, pps, waves):
                smax = 4 * th + 3
                pss = {}
                for pp in pps:
                    pss[(pp, 0)] = pmm.tile([P, H], f32, tag="pmm",
                                            name=f"psd{pp}_0")
                    pss[(pp, 1)] = pmm.tile([P, H], f32, tag="pmm",
                                            name=f"psd{pp}_1")
                for wave in waves:
                    for pp in pps:
                        for zi, pq in ((0, pp), (1, pp + 4)):
                            ps = pss[(pp, zi)]
                            for sj in wave:
                                off = max(0, sj * P - th * H)
                                nc.tensor.matmul(
                                    ps[:, off:H],
                                    v_all[:, sj, pq * P:(pq + 1) * P],
                                    spw[:, 4 * th + sj, off:H],
                                    start=(sj == 0), stop=(sj == smax))
                for pp in pps:
                    hs = slice(th * H, (th + 1) * H)
                    cs = cos_c[:, pp, hs]
                    sn = sin_c[:, pp, hs]
                    # evacuate psum straight into ro, then rotate in place;
                    # the four products are read into scratch before the
                    # overwrites, split gpsimd/DVE so neither gates phase D
                    oz0 = ro[:, pp, hs]
                    oz1 = ro[:, pp + 4, hs]
                    # ACT is idle in the D window: let it evacuate psum
                    nc.scalar.activation(oz0, pss[(pp, 0)][:], AF.Copy)
                    nc.scalar.activation(oz1, pss[(pp, 1)][:], AF.Copy)
                    ga = g_pool.tile([P, H], f16, tag="ga", bufs=1)
                    nc.gpsimd.tensor_mul(ga[:], oz0, cs)
                    ma = m_pool.tile([P, H], f16, tag="ma", bufs=1)
                    mb = m_pool.tile([P, H], f16, tag="mb", bufs=1)
                    mc = m_pool.tile([P, H], f16, tag="mc", bufs=1)
                    nc.vector.tensor_mul(mc[:], oz1, sn)
                    nc.vector.tensor_mul(ma[:], oz0, sn)
                    nc.vector.tensor_mul(mb[:], oz1, cs)
                    nc.vector.tensor_add(oz0, ga[:], mc[:])
                    nc.vector.tensor_sub(oz1, mb[:], ma[:])

            group(0, (0, 1, 2), ((0, 1), (2, 3)))
            group(0, (3,), ((0, 1), (2, 3)))
            group(1, (0, 1, 2), ((0, 1, 2, 3), (4, 5, 6, 7)))
            group(1, (3,), ((0, 1, 2, 3), (4, 5, 6, 7)))

        def emit_phaseE(b, st, wp):
            ro = st.ro
            ci_order = [0, 1, 2, 4, 5, 6, 3, 7]
            for ti in range(NT):
                pse = [pmm.tile([P, H], f32, tag="pmm", name="pe0"),
                       pmm.tile([P, H], f32, tag="pmm", name="pe1")]
                for j, ci in enumerate(ci_order):
                    for ch in range(2):
                        # both ch halves share one LDWEIGHTS
                        nc.tensor.matmul(pse[ch][:],
                                         ro[:, ci, ti * P:(ti + 1) * P],
                                         wp[:, ci, ch * H:(ch + 1) * H],
                                         start=(j == 0),
                                         stop=(j == NCH - 1 and not has_bias))
                yt = y_pool.tile([P, C], f16, tag="y")
                for ch in range(2):
                    if has_bias:
                        nc.tensor.matmul(pse[ch][:], ones_row[:, :P],
                                         brows[:, 3, ch * H:(ch + 1) * H],
                                         start=False, stop=True)
                    nc.scalar.activation(yt[:, ch * H:(ch + 1) * H],
                                         pse[ch][:], AF.Copy)
                # one big store per ti, alternating queues so the store
                # stream drains inside phase E instead of tailing after it
                eng = nc.sync if ti % 2 == 0 else nc.gpsimd
                eng.dma_start(y_d[b, ti * P:(ti + 1) * P, :], yt[:])

        # ================= schedule =================
        for b in range(BPC):
            st = states[b]
            emit_vproj_and_phaseA(b, st)
            # prefetch next batch's x/x8/ang while PE chews on k-proj;
            # sync and gpsimd queues are otherwise idle here
            if b + 1 < BPC:
                nxt = states[b + 1]
                nxt.x_all = dma_x(b + 1)
                nxt.x8 = dma_x8(b + 1, nc.sync)
            st.k16 = emit_proj(b, st, wk8, k_pool, "k", 0)
            if b + 1 < BPC:
                nxt.a_all = dma_ang(b + 1, nc.gpsimd)
            st.q16 = emit_proj(b, st, wq8, q_pool, "q", 1)
            emit_vrot(b, st)
            emit_phaseC(b, st)
            emit_phaseD(b, st)
            emit_phaseE(b, st, wp)

    nc.compile()
    return nc


def _get_program(has_bias: bool):
    key = ("prog3", has_bias)
    if key not in _CACHE:
        _CACHE[key] = _build(has_bias)
    return _CACHE[key]


def _prep_host(x, idx, Wk, Wq, Wv, Wp, ang_emb, biases):
    import ml_dtypes
    e4 = ml_dtypes.float8_e4m3
    perm = np.concatenate([np.arange(0, C, 2), np.arange(1, C, 2)])
    # x: [B, T, C] -> per batch [P, NCH, T] (partition-major chunks of x^T)
    xT = np.transpose(np.asarray(x, np.float32), (0, 2, 1))      # [B, C, T]
    xTt = xT.reshape(B, NCH, P, T)
    xTt = np.ascontiguousarray(np.transpose(xTt, (0, 2, 1, 3)))
    xT16 = xTt.astype(np.float16).reshape(NCORES, BPC, P, NCH, T)
    xT8 = xTt.astype(e4).reshape(NCORES, BPC, P, NCH, T)
    idx = np.asarray(idx).astype(np.int64)
    ang = np.asarray(ang_emb, np.float32)[idx]                   # [B, T, D]
    ang16 = ang.astype(np.float16).reshape(B, NT, P, D)
    ang16 = np.ascontiguousarray(np.transpose(ang16, (0, 2, 1, 3)))
    ang16 = ang16.reshape(NCORES, BPC, P, NT, D)

    def wtile(m, dtype=np.float16):
        w = np.ascontiguousarray(m).astype(dtype).reshape(NCH, P, C)
        return np.ascontiguousarray(np.transpose(w, (1, 0, 2)))

    # k/q weights carry x32 so fp8e4 values sit in the normal range;
    # folded out by the Exp scale (1/1024) in phase C.
    wk8T = wtile(np.asarray(Wk, np.float32)[perm].T * 32.0, e4)
    wq8T = wtile(np.asarray(Wq, np.float32)[perm].T * 32.0, e4)
    wvT = wtile(np.asarray(Wv, np.float32)[perm].T)
    wpT = wtile(np.asarray(Wp, np.float32)[:, perm].T)

    vmat = (np.arange(1408)[None, :] <= np.arange(P)[:, None] + 896)
    vmat = vmat.astype(np.float16)
    triu = np.triu(np.ones((P, P), np.float16))

    consts = dict(wk8T=wk8T, wq8T=wq8T, wvT=wvT, wpT=wpT, vmat=vmat, triu=triu)
    bk, bq, bv, bp = (np.asarray(b_, np.float32) for b_ in biases)
    has_bias = any(np.any(b_ != 0) for b_ in (bk, bq, bv, bp))
    if has_bias:
        brows = np.stack([bk[perm] * 32.0, bq[perm] * 32.0, bv[perm],
                          bp]).astype(np.float16)
        consts["biases"] = brows.reshape(4, 1, C)
    return xT16, xT8, ang16, consts, has_bias


def run_on_device(x, idx, Wk, Wq, Wv, Wp, ang_emb, biases, trace=False):
    _install_profile_hook()
    import concourse.bass_utils as bass_utils
    bass_utils.upload_artifacts = lambda tmpdir: "local://" + tmpdir
    from concourse.bass_utils import run_bass_kernel_spmd

    xT16, xT8, ang16, consts, has_bias = _prep_host(x, idx, Wk, Wq, Wv, Wp,
                                                    ang_emb, biases)
    nc = _get_program(has_bias)
    in_maps = []
    for c in range(NCORES):
        m = {"xT": xT16[c], "x8": xT8[c], "ang": ang16[c]}
        m.update(consts)
        in_maps.append(m)
    res = run_bass_kernel_spmd(nc, in_maps, list(range(NCORES)), trace=trace)
    y = np.empty((B, T, C), np.float32)
    for c in range(NCORES):
        y[c * BPC:(c + 1) * BPC] = res.results[c]["y"].astype(np.float32)
    return y, res


def kernel(x, idx, Wk, bk, Wq, bq, Wv, bv, Wp, bp, ang_emb):
    y, _ = run_on_device(x, idx, Wk, Wq, Wv, Wp, ang_emb, (bk, bq, bv, bp))
    return y



# revision 33
# speedup vs baseline: 1.2699x; 1.0094x over previous
"""Trainium2 Bass kernel for CumsumAttention (v3).

Full-input contract: kernel(**inputs) takes the complete (unsharded) inputs
and returns the full [B, T, C] float32 output. Internally the work is
data-parallel over the batch dimension across 8 NeuronCores (2 batches per
core); each core runs the same Bass/Tile program on its own batch shard.

Math (per batch, reference semantics):
  k = x @ Wk.T ; q = x @ Wq.T ; v = x @ Wv.T   (biases all zero here)
  angle[t] = sum_{s>=t} ang_emb[idx[s]]        (reverse cumsum over T)
  rot/inv_rot: per channel-pair rotation by angle
  wei  = softplus((rot(k) @ rot(q).T) / sqrt(C)) masked causally (s <= t)
  out  = inv_rot(wei @ rot(v)) @ Wp.T

Engine plan (vs the 337.5us fp16 baseline):
  - Logits matmul in fp8e4 DoubleRow (2 K-tiles per instruction, 2x MAC
    rate); rot(k)/rot(q) written to fp8 by the DVE rotation.  Measured
    rel err ~1.3e-2 < 2e-2.
  - v-projection accumulates ci-outer across PSUM-resident groups so
    compute starts on the first x/Wv chunk; the angle-cumsum PE chain is
    interleaved one tile per chunk so its carry round-trips hide.
  - PSUM evacuation for the v-projection and attention-output goes to
    GpSimd, keeping ACT free for the serial Exp/Ln softplus chain that
    gates phase D; phase D consumes softplus slices in completion order
    via sj-split waves.
  - Causal masking runs in exp-space between Exp and Ln (ln(0+1)=0), so
    diagonal-block masks overlap the off-diagonal Ln chain.
  - DMA: one descriptor only sustains ~50GB/s and costs ~0.6us of issue
    on its engine, so loads are chunked across parallel queues and issued
    from sync/scalar/gpsimd; batch b+1's x/Wv/ang/Wk are prefetched
    during batch b's attention phases at points where the issuing engine
    is idle.
  - y is stored fp16 (2MB/batch) and upcast on host.
"""

import sys
import types
from contextlib import ExitStack

import numpy as np

if "/opt/trn_rl_repo" not in sys.path:
    sys.path.insert(0, "/opt/trn_rl_repo")

B, T, C = 16, 1024, 1024
D = C // 2
NCORES = 8
BPC = B // NCORES          # batches per core
P = 128                    # partitions
NT = T // P                # t tiles
NCH = C // P               # c tiles
ND = D // P                # d tiles (channel pairs)
H = 512                    # matmul free-dim block
PI = float(np.pi)
SCALE = float(C ** -0.5)

_CACHE = {}


def _install_profile_hook():
    """Register the axon NTFF profile hook if the image's antenv lacks it."""
    try:
        import antenv
        from trn_agent_boot.trn_boot import _ntff_profile_via_ctypes
    except Exception:
        return
    if "antenv.axon_hooks" in sys.modules:
        return
    try:
        hook = _ntff_profile_via_ctypes("/opt/axon/libaxon_pjrt.so")
    except Exception:
        return
    mod = types.ModuleType("antenv.axon_hooks")
    mod.get_axon_ntff_profile_hook = lambda: hook
    mod.set_axon_ntff_profile_hook = lambda h: None
    sys.modules["antenv.axon_hooks"] = mod
    antenv.axon_hooks = mod


def _build(has_bias: bool):
    import concourse.bass as bass  # noqa: F401
    import concourse.mybir as mybir
    import concourse.tile as tile
    from concourse import bacc
    from concourse.masks import make_identity

    dt = mybir.dt
    AF = mybir.ActivationFunctionType
    f16 = dt.float16
    f32 = dt.float32
    f8 = dt.float8e4
    DR = mybir.MatmulPerfMode.DoubleRow

    # Keep Exp/Ln in one table set and Sin in trig_and_small so the program
    # avoids mid-phase ACT table switches (a switch costs ~1.3us).
    import concourse.hw_specs as _hw_specs
    if not hasattr(_hw_specs, "_orig_get_activation_tables"):
        _hw_specs._orig_get_activation_tables = _hw_specs.get_activation_tables

        def _filtered_tables(arch):
            tabs = _hw_specs._orig_get_activation_tables(arch)
            for name, fns in tabs.items():
                if name != "natural_log_exp_and_others":
                    fns.discard(AF.Exp)
                    fns.discard(AF.Ln)
                if name != "trig_and_small":
                    fns.discard(AF.Sin)
            return tabs

        _hw_specs.get_activation_tables = _filtered_tables
        bacc.get_activation_tables = _filtered_tables

    nc = bacc.Bacc("TRN2", target_bir_lowering=False, debug=False,
                   num_devices=NCORES)

    xT_d = nc.dram_tensor("xT", [BPC, P, NCH, T], f16, kind="ExternalInput").ap()
    x8_d = nc.dram_tensor("x8", [BPC, P, NCH, T], f8, kind="ExternalInput").ap()
    ang_d = nc.dram_tensor("ang", [BPC, P, NT, D], f16, kind="ExternalInput").ap()
    wk_d = nc.dram_tensor("wk8T", [P, NCH, C], f8, kind="ExternalInput").ap()
    wq_d = nc.dram_tensor("wq8T", [P, NCH, C], f8, kind="ExternalInput").ap()
    wv_d = nc.dram_tensor("wvT", [P, NCH, C], f16, kind="ExternalInput").ap()
    wp_d = nc.dram_tensor("wpT", [P, NCH, C], f16, kind="ExternalInput").ap()
    vmat_d = nc.dram_tensor("vmat", [P, 1408], f16, kind="ExternalInput").ap()
    triu_d = nc.dram_tensor("triu", [P, P], f16, kind="ExternalInput").ap()
    if has_bias:
        bias_d = nc.dram_tensor("biases", [4, 1, C], f16, kind="ExternalInput").ap()
    y_d = nc.dram_tensor("y", [BPC, T, C], f16, kind="ExternalOutput").ap()

    with tile.TileContext(nc) as tc, ExitStack() as ctx:
        const = ctx.enter_context(tc.tile_pool(name="const", bufs=1))
        wv_pool = ctx.enter_context(tc.tile_pool(name="wvpool", bufs=1))
        wpr_pool = ctx.enter_context(tc.tile_pool(name="wprpool", bufs=1))
        w8_pool = ctx.enter_context(tc.tile_pool(name="w8pool", bufs=2))
        x8_pool = ctx.enter_context(tc.tile_pool(name="x8pool", bufs=1))
        xo_pool = ctx.enter_context(tc.tile_pool(name="xopool", bufs=2))
        a_pool = ctx.enter_context(tc.tile_pool(name="apool", bufs=1))
        st_pool = ctx.enter_context(tc.tile_pool(name="stpool", bufs=1))
        ct_pool = ctx.enter_context(tc.tile_pool(name="ctpool", bufs=1))
        sc_pool = ctx.enter_context(tc.tile_pool(name="scpool", bufs=1))
        cc_pool = ctx.enter_context(tc.tile_pool(name="ccpool", bufs=1))
        k_pool = ctx.enter_context(tc.tile_pool(name="kpool", bufs=1))
        q_pool = ctx.enter_context(tc.tile_pool(name="qpool", bufs=1))
        v_pool = ctx.enter_context(tc.tile_pool(name="vpool", bufs=1))
        spw_pool = ctx.enter_context(tc.tile_pool(name="spwpool", bufs=1))
        m_pool = ctx.enter_context(tc.tile_pool(name="mpool", bufs=2))
        g_pool = ctx.enter_context(tc.tile_pool(name="gpool", bufs=2))
        sp_pool = ctx.enter_context(tc.tile_pool(name="sppool", bufs=2))
        y_pool = ctx.enter_context(tc.tile_pool(name="ypool", bufs=2))
        pmm = ctx.enter_context(tc.tile_pool(name="pmm", bufs=8, space="PSUM"))

        # ---- batch-0 input DMA first: engines are idle, land ASAP.
        # HWDGE queues are FIFO with ~2us fixed cost per dma_start, so use
        # few LARGE transfers (small first chunk to unblock the first MM).
        def dma_x(b):
            x_all = xo_pool.tile([P, NCH, T], f16, tag="xo")
            for cj in range(4):
                nc.sync.dma_start(x_all[:, 2 * cj:2 * cj + 2],
                                  xT_d[b, :, 2 * cj:2 * cj + 2])
            return x_all

        def dma_x8(b, engine):
            x8 = x8_pool.tile([P, NCH, T], f8, tag="x8")
            engine.dma_start(x8[:], x8_d[b])
            return x8

        def dma_w8(engine):
            wk8 = w8_pool.tile([P, NCH, C], f8, tag="w8")
            wq8 = w8_pool.tile([P, NCH, C], f8, tag="w8")
            engine.dma_start(wk8[:], wk_d[:])
            engine.dma_start(wq8[:], wq_d[:])
            return wk8, wq8

        def dma_wv(engine):
            wv = wv_pool.tile([P, NCH, C], f16, tag="wv")
            engine.dma_start(wv[:, 0:2], wv_d[:, 0:2])
            engine.dma_start(wv[:, 2:NCH], wv_d[:, 2:NCH])
            return wv

        def dma_ang(b, engine):
            a_all = a_pool.tile([P, NT, D], f16, tag="a")
            engine.dma_start(a_all[:, 4:NT], ang_d[b, :, 4:NT])
            engine.dma_start(a_all[:, 0:4], ang_d[b, :, 0:4])
            return a_all

        def dma_wp(engine):
            w_sb = wpr_pool.tile([P, NCH, C], f16, tag="w")
            engine.dma_start(w_sb[:], wp_d[:])
            return w_sb

        class S:
            pass

        states = [S() for _ in range(BPC)]
        st0 = states[0]
        # ang first: the angle cumsums only need ang+vmat, so they start
        # while the bulkier x/wv streams are still in flight.  Everything
        # the projections need later trails on the gpsimd queue (FIFO).
        st0.x_all = dma_x(0)
        wv = dma_wv(nc.gpsimd)
        gate = const.tile([1, 8], f16)
        gate2 = const.tile([1, 8], f16)
        # cumsums run after the v-pass, so ang/vmat are not start-critical:
        # gate them behind the first x chunk to give the v-pass streams the
        # full HBM bandwidth at kernel start
        nc.scalar.activation(gate2[:], st0.x_all[0:1, 0, 0:8], AF.Copy)
        st0.a_all = dma_ang(0, nc.scalar)

        # consts: vmat[r, u] = 1 iff u <= r + 896 encodes every s>=t step
        # block (its [896:1024] slice is tril).
        vmat = const.tile([P, 1408], f16)
        nc.scalar.dma_start(vmat[:], vmat_d[:])
        triu = const.tile([P, P], f16)
        nc.scalar.dma_start(triu[:], triu_d[:])
        # weights are shared by both batches: load once, gated behind the
        # x16 bulk so the phase-A-critical streams get full HBM bandwidth
        nc.gpsimd.tensor_copy(gate[:], st0.x_all[0:1, NCH - 1, 0:8])
        st0.x8 = dma_x8(0, nc.gpsimd)
        wk8, wq8 = dma_w8(nc.gpsimd)
        wp = dma_wp(nc.gpsimd)
        if has_bias:
            ones_row = const.tile([1, H], f16)
            nc.gpsimd.memset(ones_row[:], 1.0)
            brows = const.tile([1, 4, C], f16)
            for i in range(4):
                nc.scalar.dma_start(brows[:, i], bias_d[i])

        def emit_vproj_and_phaseA(b, st):
            x_all, a_all = st.x_all, st.a_all
            v_all = v_pool.tile([P, NT, C], f16, tag="v")
            st.v_all = v_all

            # phase-A state threaded through the interleaved emission
            sin_t = st_pool.tile([P, NT, D], f16, tag="sin_t")
            cos_t = ct_pool.tile([P, NT, D], f16, tag="cos_t")
            st.sin_t, st.cos_t = sin_t, cos_t

            def vpass(tis):
                pss = {}
                for ti in tis:
                    for ch in range(2):
                        pss[(ti, ch)] = pmm.tile([P, H], f32, tag="pmm",
                                                 name=f"psv{ti}_{ch}")
                for ci in range(NCH):
                    for ti in tis:
                        for ch in range(2):
                            nc.tensor.matmul(
                                pss[(ti, ch)][:],
                                x_all[:, ci, ti * P:(ti + 1) * P],
                                wv[:, ci, ch * H:(ch + 1) * H],
                                start=(ci == 0),
                                stop=(ci == NCH - 1 and not has_bias))
                for ti in tis:
                    for ch in range(2):
                        ps = pss[(ti, ch)]
                        if has_bias:
                            nc.tensor.matmul(ps[:], ones_row[:, :P],
                                             brows[:, 2, ch * H:(ch + 1) * H],
                                             start=False, stop=True)
                        nc.vector.tensor_copy(v_all[:, ti, ch * H:(ch + 1) * H],
                                              ps[:])

            def emit_cumsum_t():
                # t-major reverse cumsum as pure matmuls: psa[tb] =
                # sum_{sb>=tb} V(sb,tb)^T a[sb].  The stationary vmat block
                # depends only on delta = tb-sb, so the inner tb loop keeps
                # one LDWEIGHTS per delta; no serial carry round-trips.
                for half in (1, 0):
                    tbs = range(4 * half, 4 * half + 4)
                    pss = {tb: pmm.tile([P, D], f32, tag="pmm",
                                        name=f"psa{tb}") for tb in tbs}
                    for delta in range(0, -8, -1):
                        for tb in tbs:
                            sb = tb - delta
                            if sb > NT - 1:
                                continue
                            nc.tensor.matmul(
                                pss[tb][:],
                                vmat[:, (7 + delta) * P:(8 + delta) * P],
                                a_all[:, sb],
                                start=(delta == 0), stop=(sb == NT - 1))
                    for tb in tbs:
                        u = sp_pool.tile([P, D], f16, tag="sp")
                        w = sp_pool.tile([P, D], f16, tag="sp")
                        nc.vector.add_range_wrap(u[:], pss[tb][:], 0.0, PI,
                                                 2 * PI)
                        nc.vector.add_range_wrap(w[:], pss[tb][:], PI / 2, PI,
                                                 2 * PI)
                        nc.scalar.activation(sin_t[:, tb], u[:], AF.Sin)
                        nc.scalar.activation(cos_t[:, tb], w[:], AF.Sin)


            def emit_cumsum_c():
                # channel-major angle: reverse-cumsum over tokens computed
                # directly on the PE via vmat step blocks (contraction over
                # the token-tile partitions), then sin/cos per (dj, th).
                sin_c = sc_pool.tile([P, ND, T], f16, tag="sin_c")
                cos_c = cc_pool.tile([P, ND, T], f16, tag="cos_c")
                st.sin_c, st.cos_c = sin_c, cos_c
                for dj in range(ND):
                    psc = [pmm.tile([P, H], f32, tag="pmm", name=f"pcm{dj}0"),
                           pmm.tile([P, H], f32, tag="pmm", name=f"pcm{dj}1")]
                    for sb in range(NT):
                        # the a-tile stationary serves both th halves
                        for th in range(2):
                            if th == 1 and sb < 4:
                                continue  # all-zero step block (s<512<=t)
                            off = (7 - sb) * P + th * H
                            nc.tensor.matmul(psc[th][:],
                                             a_all[:, sb, dj * P:(dj + 1) * P],
                                             vmat[:, off:off + H],
                                             start=(sb == (0 if th == 0 else 4)),
                                             stop=(sb == NT - 1))
                    for th in range(2):
                        hs = slice(th * H, (th + 1) * H)
                        u = sp_pool.tile([P, H], f16, tag="sp")
                        w = sp_pool.tile([P, H], f16, tag="sp")
                        nc.vector.add_range_wrap(u[:], psc[th][:], 0.0, PI,
                                                 2 * PI)
                        nc.vector.add_range_wrap(w[:], psc[th][:], PI / 2, PI,
                                                 2 * PI)
                        nc.scalar.activation(sin_c[:, dj, hs], u[:], AF.Sin)
                        nc.scalar.activation(cos_c[:, dj, hs], w[:], AF.Sin)

            vpass((0, 1))
            vpass((2, 3))
            vpass((4, 5))
            vpass((6, 7))
            emit_cumsum_c()
            emit_cumsum_t()

        def emit_vrot(b, st):
            """Rotate v in place (t-major).  Emitted after the q-projection
            so this long DVE chain overlaps phase C's PE work instead of
            stalling the k-projection's psum evacuations."""
            v_all, sin_t, cos_t = st.v_all, st.sin_t, st.cos_t
            for tj in range(NT // 2):
                tp = slice(2 * tj, 2 * tj + 2)
                z0 = v_all[:, tp, 0:D]
                z1 = v_all[:, tp, D:C]
                cs = cos_t[:, tp]
                sn = sin_t[:, tp]
                ma = m_pool.tile([P, 2, H], f16, tag="ma", bufs=1)
                mb = m_pool.tile([P, 2, H], f16, tag="mb", bufs=1)
                mc = m_pool.tile([P, 2, H], f16, tag="mc", bufs=1)
                md = m_pool.tile([P, 2, H], f16, tag="md", bufs=1)
                nc.vector.tensor_mul(ma[:], z0, cs)
                nc.vector.tensor_mul(mb[:], z1, sn)
                nc.vector.tensor_mul(mc[:], z0, sn)
                nc.vector.tensor_mul(md[:], z1, cs)
                nc.vector.tensor_sub(z0, ma[:], mb[:])
                nc.vector.tensor_add(z1, mc[:], md[:])

        def emit_proj(b, st, w8, out_pool, tag, bias_idx):
            """k/q projection in c-major via fp8 DoubleRow (weights carry a
            x32 prescale; folded out at the Exp).  Rotation output f16."""
            x8 = st.x8
            cos_c, sin_c = st.cos_c, st.sin_c
            out16 = out_pool.tile([P, NCH, T], f16, tag=tag)
            for p in range(ND):
                # psum lands in the final tile; the rotation rewrites it in
                # place (all four products are read into scratch first), so
                # there is no z staging tile and no WAR chain between the
                # ACT evacuations of consecutive p groups.
                for zi, co in ((0, p), (1, p + 4)):
                    pst = [pmm.tile([P, H], f32, tag="pmm", name=f"pj{zi}0"),
                           pmm.tile([P, H], f32, tag="pmm", name=f"pj{zi}1")]
                    for cj in range(NCH // 2):
                        for th in range(2):
                            # both th halves share one LDWEIGHTS
                            nc.tensor.matmul(pst[th][:],
                                             w8[:, 2 * cj:2 * cj + 2,
                                                co * P:(co + 1) * P],
                                             x8[:, 2 * cj:2 * cj + 2,
                                                th * H:(th + 1) * H],
                                             start=(cj == 0),
                                             stop=(cj == NCH // 2 - 1
                                                   and not has_bias),
                                             perf_mode=DR)
                    for th in range(2):
                        if has_bias:
                            nc.tensor.matmul(pst[th][:],
                                             brows[:, bias_idx, co * P:(co + 1) * P],
                                             ones_row[:], start=False, stop=True)
                        nc.scalar.activation(out16[:, co, th * H:(th + 1) * H],
                                             pst[th][:], AF.Copy)
                cs = cos_c[:, p, :]
                sn = sin_c[:, p, :]
                z0 = out16[:, p, :]
                z1 = out16[:, p + 4, :]
                ma = m_pool.tile([P, T], f16, tag="ma", bufs=1)
                mb = m_pool.tile([P, T], f16, tag="mb", bufs=1)
                mc = m_pool.tile([P, T], f16, tag="mc", bufs=1)
                md = m_pool.tile([P, T], f16, tag="md", bufs=1)
                nc.vector.tensor_mul(ma[:], z0, cs)
                nc.vector.tensor_mul(mb[:], z1, sn)
                nc.vector.tensor_mul(mc[:], z0, sn)
                nc.vector.tensor_mul(md[:], z1, cs)
                nc.vector.tensor_sub(z0, ma[:], mb[:])
                nc.vector.tensor_add(z1, mc[:], md[:])
            return out16

        def emit_phaseC(b, st):
            """wei^T = softplus(q.k / sqrt(C)): f16 logits, Exp,
            exp-space causal mask, Ln.  The x32 weight prescale on both
            k and q is folded out via the Exp scale (1/1024)."""
            k16, q16 = st.k16, st.q16
            spw = spw_pool.tile([P, 12, H], f16, tag="spw")
            st.spw = spw
            # one pass over si: the q-block stationary serves both
            # th-halves (one LDWEIGHTS per ci).  Exp-space diagonal mask +
            # Ln run per-slice so early slices are ready for phase D fast.
            for si in range(NT):
                has0 = si < 4
                off0 = si * P
                off1 = max(0, si * P - H)
                ps0 = (pmm.tile([P, H], f32, tag="pmm", name="pc0")
                       if has0 else None)
                ps1 = pmm.tile([P, H], f32, tag="pmm", name="pc1")
                for j, ci in enumerate((0, 4, 2, 6, 1, 5, 3, 7)):
                    if has0:
                        nc.tensor.matmul(ps0[:, 0:H - off0],
                                         q16[:, ci, si * P:(si + 1) * P],
                                         k16[:, ci, off0:H],
                                         start=(j == 0), stop=(j == NCH - 1))
                    nc.tensor.matmul(ps1[:, 0:H - off1],
                                     q16[:, ci, si * P:(si + 1) * P],
                                     k16[:, ci, H + off1:T],
                                     start=(j == 0), stop=(j == NCH - 1))
                if has0:
                    nc.scalar.activation(spw[:, si, off0:H],
                                         ps0[:, 0:H - off0], AF.Exp,
                                         scale=SCALE / 1024.0)
                nc.scalar.activation(spw[:, 4 + si, off1:H],
                                     ps1[:, 0:H - off1], AF.Exp,
                                     scale=SCALE / 1024.0)
                if has0:
                    nc.vector.tensor_mul(spw[:, si, off0:off0 + P],
                                         spw[:, si, off0:off0 + P], triu[:])
                    nc.scalar.activation(spw[:, si, off0:H],
                                         spw[:, si, off0:H], AF.Ln, bias=1.0)
                else:
                    nc.vector.tensor_mul(spw[:, 4 + si, off1:off1 + P],
                                         spw[:, 4 + si, off1:off1 + P],
                                         triu[:])
                nc.scalar.activation(spw[:, 4 + si, off1:H],
                                     spw[:, 4 + si, off1:H], AF.Ln, bias=1.0)

        def emit_phaseD(b, st):
            """out^T = v.T @ wei^T, inverse-rotated -> ro.  sj-split waves:
            each psum group first accumulates the early softplus slices so
            the tail of the ACT chain is only needed late."""
            v_all, spw = st.v_all, st.spw
            cos_c, sin_c = st.cos_c, st.sin_c
            ro = xo_pool.tile([P, NCH, T], f16, tag="xo")
            st.ro = ro

            def group(th, pps, waves):
                smax = 4 * th + 3
                pss = {}
                for pp in pps:
                    pss[(pp, 0)] = pmm.tile([P, H], f32, tag="pmm",
                                            name=f"psd{pp}_0")
                    pss[(pp, 1)] = pmm.tile([P, H], f32, tag="pmm",
                                            name=f"psd{pp}_1")
                for wave in waves:
                    for pp in pps:
                        for zi, pq in ((0, pp), (1, pp + 4)):
                            ps = pss[(pp, zi)]
                            for sj in wave:
                                off = max(0, sj * P - th * H)
                                nc.tensor.matmul(
                                    ps[:, off:H],
                                    v_all[:, sj, pq * P:(pq + 1) * P],
                                    spw[:, 4 * th + sj, off:H],
                                    start=(sj == 0), stop=(sj == smax))
                for pp in pps:
                    hs = slice(th * H, (th + 1) * H)
                    cs = cos_c[:, pp, hs]
                    sn = sin_c[:, pp, hs]
                    # evacuate psum straight into ro, then rotate in place;
                    # the four products are read into scratch before the
                    # overwrites, split gpsimd/DVE so neither gates phase D
                    oz0 = ro[:, pp, hs]
                    oz1 = ro[:, pp + 4, hs]
                    # ACT is idle in the D window: let it evacuate psum
                    nc.scalar.activation(oz0, pss[(pp, 0)][:], AF.Copy)
                    nc.scalar.activation(oz1, pss[(pp, 1)][:], AF.Copy)
                    ga = g_pool.tile([P, H], f16, tag="ga", bufs=1)
                    nc.gpsimd.tensor_mul(ga[:], oz0, cs)
                    ma = m_pool.tile([P, H], f16, tag="ma", bufs=1)
                    mb = m_pool.tile([P, H], f16, tag="mb", bufs=1)
                    mc = m_pool.tile([P, H], f16, tag="mc", bufs=1)
                    nc.vector.tensor_mul(mc[:], oz1, sn)
                    nc.vector.tensor_mul(ma[:], oz0, sn)
                    nc.vector.tensor_mul(mb[:], oz1, cs)
                    nc.vector.tensor_add(oz0, ga[:], mc[:])
                    nc.vector.tensor_sub(oz1, mb[:], ma[:])

            group(0, (0, 1, 2), ((0, 1), (2, 3)))
            group(0, (3,), ((0, 1), (2, 3)))
            group(1, (0, 1, 2), ((0, 1, 2, 3), (4, 5, 6, 7)))
            group(1, (3,), ((0, 1, 2, 3), (4, 5, 6, 7)))

        def emit_phaseE(b, st, wp):
            ro = st.ro
            ci_order = [0, 1, 2, 4, 5, 6, 3, 7]
            for ti in range(NT):
                pse = [pmm.tile([P, H], f32, tag="pmm", name="pe0"),
                       pmm.tile([P, H], f32, tag="pmm", name="pe1")]
                for j, ci in enumerate(ci_order):
                    for ch in range(2):
                        # both ch halves share one LDWEIGHTS
                        nc.tensor.matmul(pse[ch][:],
                                         ro[:, ci, ti * P:(ti + 1) * P],
                                         wp[:, ci, ch * H:(ch + 1) * H],
                                         start=(j == 0),
                                         stop=(j == NCH - 1 and not has_bias))
                yt = y_pool.tile([P, C], f16, tag="y")
                for ch in range(2):
                    if has_bias:
                        nc.tensor.matmul(pse[ch][:], ones_row[:, :P],
                                         brows[:, 3, ch * H:(ch + 1) * H],
                                         start=False, stop=True)
                    nc.scalar.activation(yt[:, ch * H:(ch + 1) * H],
                                         pse[ch][:], AF.Copy)
                # one big store per ti, alternating queues so the store
                # stream drains inside phase E instead of tailing after it
                eng = nc.sync if ti % 2 == 0 else nc.gpsimd
                eng.dma_start(y_d[b, ti * P:(ti + 1) * P, :], yt[:])

        # ================= schedule =================
        for b in range(BPC):
            st = states[b]
            emit_vproj_and_phaseA(b, st)
            # prefetch next batch's x/x8/ang while PE chews on k-proj;
            # sync and gpsimd queues are otherwise idle here
            if b + 1 < BPC:
                nxt = states[b + 1]
                nxt.x_all = dma_x(b + 1)
                nxt.x8 = dma_x8(b + 1, nc.sync)
            st.k16 = emit_proj(b, st, wk8, k_pool, "k", 0)
            if b + 1 < BPC:
                nxt.a_all = dma_ang(b + 1, nc.gpsimd)
            st.q16 = emit_proj(b, st, wq8, q_pool, "q", 1)
            emit_vrot(b, st)
            emit_phaseC(b, st)
            emit_phaseD(b, st)
            emit_phaseE(b, st, wp)

    nc.compile()
    return nc


def _get_program(has_bias: bool):
    key = ("prog3", has_bias)
    if key not in _CACHE:
        _CACHE[key] = _build(has_bias)
    return _CACHE[key]


def _prep_host(x, idx, Wk, Wq, Wv, Wp, ang_emb, biases):
    import ml_dtypes
    e4 = ml_dtypes.float8_e4m3
    perm = np.concatenate([np.arange(0, C, 2), np.arange(1, C, 2)])
    # x: [B, T, C] -> per batch [P, NCH, T] (partition-major chunks of x^T)
    xT = np.transpose(np.asarray(x, np.float32), (0, 2, 1))      # [B, C, T]
    xTt = xT.reshape(B, NCH, P, T)
    xTt = np.ascontiguousarray(np.transpose(xTt, (0, 2, 1, 3)))
    xT16 = xTt.astype(np.float16).reshape(NCORES, BPC, P, NCH, T)
    xT8 = xTt.astype(e4).reshape(NCORES, BPC, P, NCH, T)
    idx = np.asarray(idx).astype(np.int64)
    ang = np.asarray(ang_emb, np.float32)[idx]                   # [B, T, D]
    ang16 = ang.astype(np.float16).reshape(B, NT, P, D)
    ang16 = np.ascontiguousarray(np.transpose(ang16, (0, 2, 1, 3)))
    ang16 = ang16.reshape(NCORES, BPC, P, NT, D)

    def wtile(m, dtype=np.float16):
        w = np.ascontiguousarray(m).astype(dtype).reshape(NCH, P, C)
        return np.ascontiguousarray(np.transpose(w, (1, 0, 2)))

    # k/q weights carry x32 so fp8e4 values sit in the normal range;
    # folded out by the Exp scale (1/1024) in phase C.
    wk8T = wtile(np.asarray(Wk, np.float32)[perm].T * 32.0, e4)
    wq8T = wtile(np.asarray(Wq, np.float32)[perm].T * 32.0, e4)
    wvT = wtile(np.asarray(Wv, np.float32)[perm].T)
    wpT = wtile(np.asarray(Wp, np.float32)[:, perm].T)

    vmat = (np.arange(1408)[None, :] <= np.arange(P)[:, None] + 896)
    vmat = vmat.astype(np.float16)
    triu = np.triu(np.ones((P, P), np.float16))

    consts = dict(wk8T=wk8T, wq8T=wq8T, wvT=wvT, wpT=wpT, vmat=vmat, triu=triu)
    bk, bq, bv, bp = (np.asarray(b_, np.float32) for b_ in biases)
    has_bias = any(np.any(b_ != 0) for b_ in (bk, bq, bv, bp))
    if has_bias:
        brows = np.stack([bk[perm] * 32.0, bq[perm] * 32.0, bv[perm],
                          bp]).astype(np.float16)
        consts["biases"] = brows.reshape(4, 1, C)
    return xT16, xT8, ang16, consts, has_bias


def run_on_device(x, idx, Wk, Wq, Wv, Wp, ang_emb, biases, trace=False):
    _install_profile_hook()
    import concourse.bass_utils as bass_utils
    bass_utils.upload_artifacts = lambda tmpdir: "local://" + tmpdir
    from concourse.bass_utils import run_bass_kernel_spmd

    xT16, xT8, ang16, consts, has_bias = _prep_host(x, idx, Wk, Wq, Wv, Wp,
                                                    ang_emb, biases)
    nc = _get_program(has_bias)
    in_maps = []
    for c in range(NCORES):
        m = {"xT": xT16[c], "x8": xT8[c], "ang": ang16[c]}
        m.update(consts)
        in_maps.append(m)
    res = run_bass_kernel_spmd(nc, in_maps, list(range(NCORES)), trace=trace)
    y = np.empty((B, T, C), np.float32)
    for c in range(NCORES):
        y[c * BPC:(c + 1) * BPC] = res.results[c]["y"].astype(np.float32)
    return y, res


def kernel(x, idx, Wk, bk, Wq, bq, Wv, bv, Wp, bp, ang_emb):
    y, _ = run_on_device(x, idx, Wk, Wq, Wv, Wp, ang_emb, (bk, bq, bv, bp))
    return y



# revision 34
# speedup vs baseline: 1.2869x; 1.0133x over previous
"""Trainium2 Bass kernel for CumsumAttention (v3).

Full-input contract: kernel(**inputs) takes the complete (unsharded) inputs
and returns the full [B, T, C] float32 output. Internally the work is
data-parallel over the batch dimension across 8 NeuronCores (2 batches per
core); each core runs the same Bass/Tile program on its own batch shard.

Math (per batch, reference semantics):
  k = x @ Wk.T ; q = x @ Wq.T ; v = x @ Wv.T   (biases all zero here)
  angle[t] = sum_{s>=t} ang_emb[idx[s]]        (reverse cumsum over T)
  rot/inv_rot: per channel-pair rotation by angle
  wei  = softplus((rot(k) @ rot(q).T) / sqrt(C)) masked causally (s <= t)
  out  = inv_rot(wei @ rot(v)) @ Wp.T

Engine plan (vs the ~332us v3 baseline; measured 322us, rel err 1.89e-2):
  - k/q projections in fp8e4 DoubleRow (x8 and 32x-prescaled Wk/Wq fp8
    shipped from host; the 1/1024 descale rides the Exp scale).  The
    error budget only allows fp8 on the k/q side: v/Wp/logits-adjacent
    fp8 all blow past the 2e-2 gate (verified in precision_sim.py), so
    the logits matmul itself runs f16 on unquantized rotated k/q.
  - Every matmul pairs two instructions per LDWEIGHTS (th/ch halves or
    si-slices share the stationary operand) - an unpaired stationary
    costs ~145ns serial LDWEIGHTS per MM.
  - Both angle layouts (t-major for the v rotation, c-major for the
    k/q/out rotations) are computed directly on the PE as reverse-cumsum
    matmuls against `vmat` step blocks (vmat[r,u] = u <= r+896 holds
    every s>=t block incl. tril); no PE transposes, no serial carry
    chain head-blocking the PE queue.
  - Rotations evacuate PSUM straight into their destination tensor and
    rotate in place (products to scratch first), removing staging tiles
    and the ACT<->DVE WAR serialization between consecutive groups.
  - DVE is co-critical with the PE: the long v-rotation chain is emitted
    after the q-projection so it overlaps phase C's PE work.  GpSimd
    tensor ops concurrent with DVE on nearby SBUF inflate DVE ops ~3x
    (port contention), so gpsimd only carries one isolated mul per
    phase-D group (own pool) plus DMA issue; phase-D PSUM evacuation
    goes to ACT, which is idle in that window.
  - Causal masking runs in exp-space between Exp and Ln (ln(0+1)=0),
    interleaved per-slice so phase D's early waves start sooner.
  - DMA: HWDGE/SWDGE queues are FIFO with ~2us fixed cost per dma_start
    and all rings share ~350GB/s, so transfers are few and large, the
    start-critical x16/wv stream first (x16 in 2-ci chunks to pipeline
    the ci-outer v-pass), ang/vmat and the x8/w8/wp pile are gated
    behind the first x chunks, weights load once for both batches, and
    y stores alternate sync/gpsimd queues to drain inside phase E.
"""

import sys
import types
from contextlib import ExitStack

import numpy as np

if "/opt/trn_rl_repo" not in sys.path:
    sys.path.insert(0, "/opt/trn_rl_repo")

B, T, C = 16, 1024, 1024
D = C // 2
NCORES = 8
BPC = B // NCORES          # batches per core
P = 128                    # partitions
NT = T // P                # t tiles
NCH = C // P               # c tiles
ND = D // P                # d tiles (channel pairs)
H = 512                    # matmul free-dim block
PI = float(np.pi)
SCALE = float(C ** -0.5)

_CACHE = {}


def _install_profile_hook():
    """Register the axon NTFF profile hook if the image's antenv lacks it."""
    try:
        import antenv
        from trn_agent_boot.trn_boot import _ntff_profile_via_ctypes
    except Exception:
        return
    if "antenv.axon_hooks" in sys.modules:
        return
    try:
        hook = _ntff_profile_via_ctypes("/opt/axon/libaxon_pjrt.so")
    except Exception:
        return
    mod = types.ModuleType("antenv.axon_hooks")
    mod.get_axon_ntff_profile_hook = lambda: hook
    mod.set_axon_ntff_profile_hook = lambda h: None
    sys.modules["antenv.axon_hooks"] = mod
    antenv.axon_hooks = mod


def _build(has_bias: bool):
    import concourse.bass as bass  # noqa: F401
    import concourse.mybir as mybir
    import concourse.tile as tile
    from concourse import bacc
    from concourse.masks import make_identity

    dt = mybir.dt
    AF = mybir.ActivationFunctionType
    f16 = dt.float16
    f32 = dt.float32
    f8 = dt.float8e4
    DR = mybir.MatmulPerfMode.DoubleRow

    # Keep Exp/Ln in one table set and Sin in trig_and_small so the program
    # avoids mid-phase ACT table switches (a switch costs ~1.3us).
    import concourse.hw_specs as _hw_specs
    if not hasattr(_hw_specs, "_orig_get_activation_tables"):
        _hw_specs._orig_get_activation_tables = _hw_specs.get_activation_tables

        def _filtered_tables(arch):
            tabs = _hw_specs._orig_get_activation_tables(arch)
            for name, fns in tabs.items():
                if name != "natural_log_exp_and_others":
                    fns.discard(AF.Exp)
                    fns.discard(AF.Ln)
                if name != "trig_and_small":
                    fns.discard(AF.Sin)
            return tabs

        _hw_specs.get_activation_tables = _filtered_tables
        bacc.get_activation_tables = _filtered_tables

    nc = bacc.Bacc("TRN2", target_bir_lowering=False, debug=False,
                   num_devices=NCORES)

    xT_d = nc.dram_tensor("xT", [BPC, P, NCH, T], f16, kind="ExternalInput").ap()
    x8_d = nc.dram_tensor("x8", [BPC, P, NCH, T], f8, kind="ExternalInput").ap()
    ang_d = nc.dram_tensor("ang", [BPC, P, NT, D], f16, kind="ExternalInput").ap()
    wk_d = nc.dram_tensor("wk8T", [P, NCH, C], f8, kind="ExternalInput").ap()
    wq_d = nc.dram_tensor("wq8T", [P, NCH, C], f8, kind="ExternalInput").ap()
    wv_d = nc.dram_tensor("wvT", [P, NCH, C], f16, kind="ExternalInput").ap()
    wp_d = nc.dram_tensor("wpT", [P, NCH, C], f16, kind="ExternalInput").ap()
    vmat_d = nc.dram_tensor("vmat", [P, 1408], f16, kind="ExternalInput").ap()
    triu_d = nc.dram_tensor("triu", [P, P], f16, kind="ExternalInput").ap()
    if has_bias:
        bias_d = nc.dram_tensor("biases", [4, 1, C], f16, kind="ExternalInput").ap()
    y_d = nc.dram_tensor("y", [BPC, T, C], f16, kind="ExternalOutput").ap()

    with tile.TileContext(nc) as tc, ExitStack() as ctx:
        const = ctx.enter_context(tc.tile_pool(name="const", bufs=1))
        wv_pool = ctx.enter_context(tc.tile_pool(name="wvpool", bufs=1))
        wpr_pool = ctx.enter_context(tc.tile_pool(name="wprpool", bufs=1))
        w8_pool = ctx.enter_context(tc.tile_pool(name="w8pool", bufs=2))
        x8_pool = ctx.enter_context(tc.tile_pool(name="x8pool", bufs=1))
        xo_pool = ctx.enter_context(tc.tile_pool(name="xopool", bufs=2))
        a_pool = ctx.enter_context(tc.tile_pool(name="apool", bufs=1))
        st_pool = ctx.enter_context(tc.tile_pool(name="stpool", bufs=1))
        ct_pool = ctx.enter_context(tc.tile_pool(name="ctpool", bufs=1))
        sc_pool = ctx.enter_context(tc.tile_pool(name="scpool", bufs=1))
        cc_pool = ctx.enter_context(tc.tile_pool(name="ccpool", bufs=1))
        k_pool = ctx.enter_context(tc.tile_pool(name="kpool", bufs=1))
        q_pool = ctx.enter_context(tc.tile_pool(name="qpool", bufs=1))
        v_pool = ctx.enter_context(tc.tile_pool(name="vpool", bufs=1))
        spw_pool = ctx.enter_context(tc.tile_pool(name="spwpool", bufs=1))
        m_pool = ctx.enter_context(tc.tile_pool(name="mpool", bufs=2))
        g_pool = ctx.enter_context(tc.tile_pool(name="gpool", bufs=2))
        sp_pool = ctx.enter_context(tc.tile_pool(name="sppool", bufs=2))
        y_pool = ctx.enter_context(tc.tile_pool(name="ypool", bufs=2))
        pmm = ctx.enter_context(tc.tile_pool(name="pmm", bufs=8, space="PSUM"))

        # ---- batch-0 input DMA first: engines are idle, land ASAP.
        # HWDGE queues are FIFO with ~2us fixed cost per dma_start, so use
        # few LARGE transfers (small first chunk to unblock the first MM).
        def dma_x(b):
            x_all = xo_pool.tile([P, NCH, T], f16, tag="xo")
            for cj in range(4):
                nc.sync.dma_start(x_all[:, 2 * cj:2 * cj + 2],
                                  xT_d[b, :, 2 * cj:2 * cj + 2])
            return x_all

        def dma_x8(b, engine):
            x8 = x8_pool.tile([P, NCH, T], f8, tag="x8")
            engine.dma_start(x8[:], x8_d[b])
            return x8

        def dma_w8(engine):
            wk8 = w8_pool.tile([P, NCH, C], f8, tag="w8")
            wq8 = w8_pool.tile([P, NCH, C], f8, tag="w8")
            engine.dma_start(wk8[:], wk_d[:])
            engine.dma_start(wq8[:], wq_d[:])
            return wk8, wq8

        def dma_wv(engine):
            wv = wv_pool.tile([P, NCH, C], f16, tag="wv")
            engine.dma_start(wv[:, 0:2], wv_d[:, 0:2])
            engine.dma_start(wv[:, 2:NCH], wv_d[:, 2:NCH])
            return wv

        def dma_ang(b, engine):
            a_all = a_pool.tile([P, NT, D], f16, tag="a")
            engine.dma_start(a_all[:, 4:NT], ang_d[b, :, 4:NT])
            engine.dma_start(a_all[:, 0:4], ang_d[b, :, 0:4])
            return a_all

        def dma_wp(engine):
            w_sb = wpr_pool.tile([P, NCH, C], f16, tag="w")
            engine.dma_start(w_sb[:], wp_d[:])
            return w_sb

        class S:
            pass

        states = [S() for _ in range(BPC)]
        st0 = states[0]
        # ang first: the angle cumsums only need ang+vmat, so they start
        # while the bulkier x/wv streams are still in flight.  Everything
        # the projections need later trails on the gpsimd queue (FIFO).
        st0.x_all = dma_x(0)
        wv = dma_wv(nc.gpsimd)
        gate = const.tile([1, 8], f16)
        gate2 = const.tile([1, 8], f16)
        # cumsums run after the v-pass, so ang/vmat are not start-critical:
        # gate them behind the first x chunk to give the v-pass streams the
        # full HBM bandwidth at kernel start
        nc.scalar.activation(gate2[:], st0.x_all[0:1, 0, 0:8], AF.Copy)
        st0.a_all = dma_ang(0, nc.scalar)

        # consts: vmat[r, u] = 1 iff u <= r + 896 encodes every s>=t step
        # block (its [896:1024] slice is tril).
        vmat = const.tile([P, 1408], f16)
        nc.scalar.dma_start(vmat[:], vmat_d[:])
        triu = const.tile([P, P], f16)
        nc.scalar.dma_start(triu[:], triu_d[:])
        # weights are shared by both batches: load once, gated behind the
        # x16 bulk so the phase-A-critical streams get full HBM bandwidth
        nc.gpsimd.tensor_copy(gate[:], st0.x_all[0:1, NCH - 1, 0:8])
        st0.x8 = dma_x8(0, nc.gpsimd)
        wk8, wq8 = dma_w8(nc.gpsimd)
        wp = dma_wp(nc.gpsimd)
        if has_bias:
            ones_row = const.tile([1, H], f16)
            nc.gpsimd.memset(ones_row[:], 1.0)
            brows = const.tile([1, 4, C], f16)
            for i in range(4):
                nc.scalar.dma_start(brows[:, i], bias_d[i])

        def emit_vproj_and_phaseA(b, st):
            x_all, a_all = st.x_all, st.a_all
            v_all = v_pool.tile([P, NT, C], f16, tag="v")
            st.v_all = v_all

            # phase-A state threaded through the interleaved emission
            sin_t = st_pool.tile([P, NT, D], f16, tag="sin_t")
            cos_t = ct_pool.tile([P, NT, D], f16, tag="cos_t")
            st.sin_t, st.cos_t = sin_t, cos_t

            def vpass(tis):
                pss = {}
                for ti in tis:
                    for ch in range(2):
                        pss[(ti, ch)] = pmm.tile([P, H], f32, tag="pmm",
                                                 name=f"psv{ti}_{ch}")
                for ci in range(NCH):
                    for ti in tis:
                        for ch in range(2):
                            nc.tensor.matmul(
                                pss[(ti, ch)][:],
                                x_all[:, ci, ti * P:(ti + 1) * P],
                                wv[:, ci, ch * H:(ch + 1) * H],
                                start=(ci == 0),
                                stop=(ci == NCH - 1 and not has_bias))
                for ti in tis:
                    for ch in range(2):
                        ps = pss[(ti, ch)]
                        if has_bias:
                            nc.tensor.matmul(ps[:], ones_row[:, :P],
                                             brows[:, 2, ch * H:(ch + 1) * H],
                                             start=False, stop=True)
                        nc.vector.tensor_copy(v_all[:, ti, ch * H:(ch + 1) * H],
                                              ps[:])

            def emit_cumsum_t():
                # t-major reverse cumsum as pure matmuls: psa[tb] =
                # sum_{sb>=tb} V(sb,tb)^T a[sb].  The stationary vmat block
                # depends only on delta = tb-sb, so the inner tb loop keeps
                # one LDWEIGHTS per delta; no serial carry round-trips.
                for half in (1, 0):
                    tbs = range(4 * half, 4 * half + 4)
                    pss = {tb: pmm.tile([P, D], f32, tag="pmm",
                                        name=f"psa{tb}") for tb in tbs}
                    for delta in range(0, -8, -1):
                        for tb in tbs:
                            sb = tb - delta
                            if sb > NT - 1:
                                continue
                            nc.tensor.matmul(
                                pss[tb][:],
                                vmat[:, (7 + delta) * P:(8 + delta) * P],
                                a_all[:, sb],
                                start=(delta == 0), stop=(sb == NT - 1))
                    for tb in tbs:
                        u = sp_pool.tile([P, D], f16, tag="sp")
                        w = sp_pool.tile([P, D], f16, tag="sp")
                        nc.vector.add_range_wrap(u[:], pss[tb][:], 0.0, PI,
                                                 2 * PI)
                        nc.vector.add_range_wrap(w[:], pss[tb][:], PI / 2, PI,
                                                 2 * PI)
                        nc.scalar.activation(sin_t[:, tb], u[:], AF.Sin)
                        nc.scalar.activation(cos_t[:, tb], w[:], AF.Sin)


            def emit_cumsum_c():
                # channel-major angle: reverse-cumsum over tokens computed
                # directly on the PE via vmat step blocks (contraction over
                # the token-tile partitions), then sin/cos per (dj, th).
                sin_c = sc_pool.tile([P, ND, T], f16, tag="sin_c")
                cos_c = cc_pool.tile([P, ND, T], f16, tag="cos_c")
                st.sin_c, st.cos_c = sin_c, cos_c
                for dj in range(ND):
                    psc = [pmm.tile([P, H], f32, tag="pmm", name=f"pcm{dj}0"),
                           pmm.tile([P, H], f32, tag="pmm", name=f"pcm{dj}1")]
                    for sb in range(NT):
                        # the a-tile stationary serves both th halves
                        for th in range(2):
                            if th == 1 and sb < 4:
                                continue  # all-zero step block (s<512<=t)
                            off = (7 - sb) * P + th * H
                            nc.tensor.matmul(psc[th][:],
                                             a_all[:, sb, dj * P:(dj + 1) * P],
                                             vmat[:, off:off + H],
                                             start=(sb == (0 if th == 0 else 4)),
                                             stop=(sb == NT - 1))
                    for th in range(2):
                        hs = slice(th * H, (th + 1) * H)
                        u = sp_pool.tile([P, H], f16, tag="sp")
                        w = sp_pool.tile([P, H], f16, tag="sp")
                        nc.vector.add_range_wrap(u[:], psc[th][:], 0.0, PI,
                                                 2 * PI)
                        nc.vector.add_range_wrap(w[:], psc[th][:], PI / 2, PI,
                                                 2 * PI)
                        nc.scalar.activation(sin_c[:, dj, hs], u[:], AF.Sin)
                        nc.scalar.activation(cos_c[:, dj, hs], w[:], AF.Sin)

            vpass((0, 1))
            vpass((2, 3))
            vpass((4, 5))
            vpass((6, 7))
            emit_cumsum_c()
            emit_cumsum_t()

        def emit_vrot(b, st):
            """Rotate v in place (t-major).  Emitted after the q-projection
            so this long DVE chain overlaps phase C's PE work instead of
            stalling the k-projection's psum evacuations."""
            v_all, sin_t, cos_t = st.v_all, st.sin_t, st.cos_t
            for tj in range(NT // 2):
                tp = slice(2 * tj, 2 * tj + 2)
                z0 = v_all[:, tp, 0:D]
                z1 = v_all[:, tp, D:C]
                cs = cos_t[:, tp]
                sn = sin_t[:, tp]
                ma = m_pool.tile([P, 2, H], f16, tag="ma", bufs=1)
                mb = m_pool.tile([P, 2, H], f16, tag="mb", bufs=1)
                mc = m_pool.tile([P, 2, H], f16, tag="mc", bufs=1)
                md = m_pool.tile([P, 2, H], f16, tag="md", bufs=1)
                nc.vector.tensor_mul(ma[:], z0, cs)
                nc.vector.tensor_mul(mb[:], z1, sn)
                nc.vector.tensor_mul(mc[:], z0, sn)
                nc.vector.tensor_mul(md[:], z1, cs)
                nc.vector.tensor_sub(z0, ma[:], mb[:])
                nc.vector.tensor_add(z1, mc[:], md[:])

        def emit_proj(b, st, w8, out_pool, tag, bias_idx):
            """k/q projection in c-major via fp8 DoubleRow (weights carry a
            x32 prescale; folded out at the Exp).  Rotation output f16."""
            x8 = st.x8
            cos_c, sin_c = st.cos_c, st.sin_c
            out16 = out_pool.tile([P, NCH, T], f16, tag=tag)
            for p in range(ND):
                # psum lands in the final tile; the rotation rewrites it in
                # place (all four products are read into scratch first), so
                # there is no z staging tile and no WAR chain between the
                # ACT evacuations of consecutive p groups.
                for zi, co in ((0, p), (1, p + 4)):
                    pst = [pmm.tile([P, H], f32, tag="pmm", name=f"pj{zi}0"),
                           pmm.tile([P, H], f32, tag="pmm", name=f"pj{zi}1")]
                    for cj in range(NCH // 2):
                        for th in range(2):
                            # both th halves share one LDWEIGHTS
                            nc.tensor.matmul(pst[th][:],
                                             w8[:, 2 * cj:2 * cj + 2,
                                                co * P:(co + 1) * P],
                                             x8[:, 2 * cj:2 * cj + 2,
                                                th * H:(th + 1) * H],
                                             start=(cj == 0),
                                             stop=(cj == NCH // 2 - 1
                                                   and not has_bias),
                                             perf_mode=DR)
                    for th in range(2):
                        if has_bias:
                            nc.tensor.matmul(pst[th][:],
                                             brows[:, bias_idx, co * P:(co + 1) * P],
                                             ones_row[:], start=False, stop=True)
                        nc.scalar.activation(out16[:, co, th * H:(th + 1) * H],
                                             pst[th][:], AF.Copy)
                cs = cos_c[:, p, :]
                sn = sin_c[:, p, :]
                z0 = out16[:, p, :]
                z1 = out16[:, p + 4, :]
                ma = m_pool.tile([P, T], f16, tag="ma", bufs=1)
                mb = m_pool.tile([P, T], f16, tag="mb", bufs=1)
                mc = m_pool.tile([P, T], f16, tag="mc", bufs=1)
                md = m_pool.tile([P, T], f16, tag="md", bufs=1)
                nc.vector.tensor_mul(ma[:], z0, cs)
                nc.vector.tensor_mul(mb[:], z1, sn)
                nc.vector.tensor_mul(mc[:], z0, sn)
                nc.vector.tensor_mul(md[:], z1, cs)
                nc.vector.tensor_sub(z0, ma[:], mb[:])
                nc.vector.tensor_add(z1, mc[:], md[:])
            return out16

        def emit_phaseC(b, st):
            """wei^T = softplus(q.k / sqrt(C)): f16 logits, Exp,
            exp-space causal mask, Ln.  The x32 weight prescale on both
            k and q is folded out via the Exp scale (1/1024)."""
            k16, q16 = st.k16, st.q16
            spw = spw_pool.tile([P, 12, H], f16, tag="spw")
            st.spw = spw
            # one pass over si: the q-block stationary serves both
            # th-halves (one LDWEIGHTS per ci).  Exp-space diagonal mask +
            # Ln run per-slice so early slices are ready for phase D fast.
            for si in range(NT):
                has0 = si < 4
                off0 = si * P
                off1 = max(0, si * P - H)
                ps0 = (pmm.tile([P, H], f32, tag="pmm", name="pc0")
                       if has0 else None)
                ps1 = pmm.tile([P, H], f32, tag="pmm", name="pc1")
                for j, ci in enumerate((0, 4, 2, 6, 1, 5, 3, 7)):
                    if has0:
                        nc.tensor.matmul(ps0[:, 0:H - off0],
                                         q16[:, ci, si * P:(si + 1) * P],
                                         k16[:, ci, off0:H],
                                         start=(j == 0), stop=(j == NCH - 1))
                    nc.tensor.matmul(ps1[:, 0:H - off1],
                                     q16[:, ci, si * P:(si + 1) * P],
                                     k16[:, ci, H + off1:T],
                                     start=(j == 0), stop=(j == NCH - 1))
                if has0:
                    nc.scalar.activation(spw[:, si, off0:H],
                                         ps0[:, 0:H - off0], AF.Exp,
                                         scale=SCALE / 1024.0)
                nc.scalar.activation(spw[:, 4 + si, off1:H],
                                     ps1[:, 0:H - off1], AF.Exp,
                                     scale=SCALE / 1024.0)
                if has0:
                    nc.vector.tensor_mul(spw[:, si, off0:off0 + P],
                                         spw[:, si, off0:off0 + P], triu[:])
                    nc.scalar.activation(spw[:, si, off0:H],
                                         spw[:, si, off0:H], AF.Ln, bias=1.0)
                else:
                    nc.vector.tensor_mul(spw[:, 4 + si, off1:off1 + P],
                                         spw[:, 4 + si, off1:off1 + P],
                                         triu[:])
                nc.scalar.activation(spw[:, 4 + si, off1:H],
                                     spw[:, 4 + si, off1:H], AF.Ln, bias=1.0)

        def emit_phaseD(b, st):
            """out^T = v.T @ wei^T, inverse-rotated -> ro.  sj-split waves:
            each psum group first accumulates the early softplus slices so
            the tail of the ACT chain is only needed late."""
            v_all, spw = st.v_all, st.spw
            cos_c, sin_c = st.cos_c, st.sin_c
            ro = xo_pool.tile([P, NCH, T], f16, tag="xo")
            st.ro = ro

            def group(th, pps, waves):
                smax = 4 * th + 3
                pss = {}
                for pp in pps:
                    pss[(pp, 0)] = pmm.tile([P, H], f32, tag="pmm",
                                            name=f"psd{pp}_0")
                    pss[(pp, 1)] = pmm.tile([P, H], f32, tag="pmm",
                                            name=f"psd{pp}_1")
                for wave in waves:
                    for pp in pps:
                        for zi, pq in ((0, pp), (1, pp + 4)):
                            ps = pss[(pp, zi)]
                            for sj in wave:
                                off = max(0, sj * P - th * H)
                                nc.tensor.matmul(
                                    ps[:, off:H],
                                    v_all[:, sj, pq * P:(pq + 1) * P],
                                    spw[:, 4 * th + sj, off:H],
                                    start=(sj == 0), stop=(sj == smax))
                for pp in pps:
                    hs = slice(th * H, (th + 1) * H)
                    cs = cos_c[:, pp, hs]
                    sn = sin_c[:, pp, hs]
                    # evacuate psum straight into ro, then rotate in place;
                    # the four products are read into scratch before the
                    # overwrites, split gpsimd/DVE so neither gates phase D
                    oz0 = ro[:, pp, hs]
                    oz1 = ro[:, pp + 4, hs]
                    # ACT is idle in the D window: let it evacuate psum
                    nc.scalar.activation(oz0, pss[(pp, 0)][:], AF.Copy)
                    nc.scalar.activation(oz1, pss[(pp, 1)][:], AF.Copy)
                    ga = g_pool.tile([P, H], f16, tag="ga", bufs=1)
                    nc.gpsimd.tensor_mul(ga[:], oz0, cs)
                    ma = m_pool.tile([P, H], f16, tag="ma", bufs=1)
                    mb = m_pool.tile([P, H], f16, tag="mb", bufs=1)
                    mc = m_pool.tile([P, H], f16, tag="mc", bufs=1)
                    nc.vector.tensor_mul(mc[:], oz1, sn)
                    nc.vector.tensor_mul(ma[:], oz0, sn)
                    nc.vector.tensor_mul(mb[:], oz1, cs)
                    nc.vector.tensor_add(oz0, ga[:], mc[:])
                    nc.vector.tensor_sub(oz1, mb[:], ma[:])

            group(0, (0, 1, 2), ((0, 1), (2, 3)))
            group(0, (3,), ((0, 1), (2, 3)))
            group(1, (0, 1, 2), ((0, 1, 2, 3), (4, 5, 6, 7)))
            group(1, (3,), ((0, 1, 2, 3), (4, 5, 6, 7)))

        def emit_phaseE(b, st, wp):
            ro = st.ro
            ci_order = [0, 1, 2, 4, 5, 6, 3, 7]
            for ti in range(NT):
                pse = [pmm.tile([P, H], f32, tag="pmm", name="pe0"),
                       pmm.tile([P, H], f32, tag="pmm", name="pe1")]
                for j, ci in enumerate(ci_order):
                    for ch in range(2):
                        # both ch halves share one LDWEIGHTS
                        nc.tensor.matmul(pse[ch][:],
                                         ro[:, ci, ti * P:(ti + 1) * P],
                                         wp[:, ci, ch * H:(ch + 1) * H],
                                         start=(j == 0),
                                         stop=(j == NCH - 1 and not has_bias))
                yt = y_pool.tile([P, C], f16, tag="y")
                for ch in range(2):
                    if has_bias:
                        nc.tensor.matmul(pse[ch][:], ones_row[:, :P],
                                         brows[:, 3, ch * H:(ch + 1) * H],
                                         start=False, stop=True)
                    nc.scalar.activation(yt[:, ch * H:(ch + 1) * H],
                                         pse[ch][:], AF.Copy)
                # one big store per ti, alternating queues so the store
                # stream drains inside phase E instead of tailing after it
                eng = nc.sync if ti % 2 == 0 else nc.gpsimd
                eng.dma_start(y_d[b, ti * P:(ti + 1) * P, :], yt[:])

        # ================= schedule =================
        for b in range(BPC):
            st = states[b]
            emit_vproj_and_phaseA(b, st)
            # prefetch next batch's x/x8/ang while PE chews on k-proj;
            # sync and gpsimd queues are otherwise idle here
            if b + 1 < BPC:
                nxt = states[b + 1]
                nxt.x_all = dma_x(b + 1)
                nxt.x8 = dma_x8(b + 1, nc.sync)
            st.k16 = emit_proj(b, st, wk8, k_pool, "k", 0)
            if b + 1 < BPC:
                nxt.a_all = dma_ang(b + 1, nc.gpsimd)
            st.q16 = emit_proj(b, st, wq8, q_pool, "q", 1)
            emit_vrot(b, st)
            emit_phaseC(b, st)
            emit_phaseD(b, st)
            emit_phaseE(b, st, wp)

    nc.compile()
    return nc


def _get_program(has_bias: bool):
    key = ("prog3", has_bias)
    if key not in _CACHE:
        _CACHE[key] = _build(has_bias)
    return _CACHE[key]


def _prep_host(x, idx, Wk, Wq, Wv, Wp, ang_emb, biases):
    import ml_dtypes
    e4 = ml_dtypes.float8_e4m3
    perm = np.concatenate([np.arange(0, C, 2), np.arange(1, C, 2)])
    # x: [B, T, C] -> per batch [P, NCH, T] (partition-major chunks of x^T)
    xT = np.transpose(np.asarray(x, np.float32), (0, 2, 1))      # [B, C, T]
    xTt = xT.reshape(B, NCH, P, T)
    xTt = np.ascontiguousarray(np.transpose(xTt, (0, 2, 1, 3)))
    xT16 = xTt.astype(np.float16).reshape(NCORES, BPC, P, NCH, T)
    xT8 = xTt.astype(e4).reshape(NCORES, BPC, P, NCH, T)
    idx = np.asarray(idx).astype(np.int64)
    ang = np.asarray(ang_emb, np.float32)[idx]                   # [B, T, D]
    ang16 = ang.astype(np.float16).reshape(B, NT, P, D)
    ang16 = np.ascontiguousarray(np.transpose(ang16, (0, 2, 1, 3)))
    ang16 = ang16.reshape(NCORES, BPC, P, NT, D)

    def wtile(m, dtype=np.float16):
        w = np.ascontiguousarray(m).astype(dtype).reshape(NCH, P, C)
        return np.ascontiguousarray(np.transpose(w, (1, 0, 2)))

    # k/q weights carry x32 so fp8e4 values sit in the normal range;
    # folded out by the Exp scale (1/1024) in phase C.
    wk8T = wtile(np.asarray(Wk, np.float32)[perm].T * 32.0, e4)
    wq8T = wtile(np.asarray(Wq, np.float32)[perm].T * 32.0, e4)
    wvT = wtile(np.asarray(Wv, np.float32)[perm].T)
    wpT = wtile(np.asarray(Wp, np.float32)[:, perm].T)

    vmat = (np.arange(1408)[None, :] <= np.arange(P)[:, None] + 896)
    vmat = vmat.astype(np.float16)
    triu = np.triu(np.ones((P, P), np.float16))

    consts = dict(wk8T=wk8T, wq8T=wq8T, wvT=wvT, wpT=wpT, vmat=vmat, triu=triu)
    bk, bq, bv, bp = (np.asarray(b_, np.float32) for b_ in biases)
    has_bias = any(np.any(b_ != 0) for b_ in (bk, bq, bv, bp))
    if has_bias:
        brows = np.stack([bk[perm] * 32.0, bq[perm] * 32.0, bv[perm],
                          bp]).astype(np.float16)
        consts["biases"] = brows.reshape(4, 1, C)
    return xT16, xT8, ang16, consts, has_bias


def run_on_device(x, idx, Wk, Wq, Wv, Wp, ang_emb, biases, trace=False):
    _install_profile_hook()
    import concourse.bass_utils as bass_utils
    bass_utils.upload_artifacts = lambda tmpdir: "local://" + tmpdir
    from concourse.bass_utils import run_bass_kernel_spmd

    xT16, xT8, ang16, consts, has_bias = _prep_host(x, idx, Wk, Wq, Wv, Wp,
                                                    ang_emb, biases)
    nc = _get_program(has_bias)
    in_maps = []
    for c in range(NCORES):
        m = {"xT": xT16[c], "x8": xT8[c], "ang": ang16[c]}
        m.update(consts)
        in_maps.append(m)
    res = run_bass_kernel_spmd(nc, in_maps, list(range(NCORES)), trace=trace)
    y = np.empty((B, T, C), np.float32)
    for c in range(NCORES):
        y[c * BPC:(c + 1) * BPC] = res.results[c]["y"].astype(np.float32)
    return y, res


def kernel(x, idx, Wk, bk, Wq, bq, Wv, bv, Wp, bp, ang_emb):
    y, _ = run_on_device(x, idx, Wk, Wq, Wv, Wp, ang_emb, (bk, bq, bv, bp))
    return y

